# revision 1
# baseline (speedup 1.0000x reference)
"""MHA (RoPE + causal softmax attention + out-proj) on 8 NeuronCores.

Sharding: DP4 x TP2. Core c: batch b = c % 4, head-group g = c // 4
(8 heads per core). Each core computes a transposed partial output
outT = (y_local @ w_o_slice^T)^T in [D, L]; host sums the two head-group
partials per batch and transposes back.

All matmuls bf16 x bf16 -> fp32 PSUM. Layout strategy:
  Phase A: qkv natural layout [L, comps] via out = xT_tile.T @ w_chunk;
           RoPE applied with strided free-dim APs straight out of PSUM;
           rotated q/k and v staged to DRAM scratch (bf16).
  Phase B: per head, q/k loaded back transposed ([comps, L]) via DMA xbar
           transpose (prefetched during phase A from a right-side pool);
           scores computed transposed (k on partitions) so the attn
           weights come out ready to be the moving operand of the attn@V
           matmul (no on-chip transposes). exp on ScalarE with the
           1/sqrt(HD) scale fused. Causal: fully-masked k-tiles skipped,
           diagonal tiles masked with 4 constant [128,512] 0/1 masks.
           Softmax denominator via all-ones [128,128] matmul accumulated
           in PSUM (M=128 so the reciprocal is already partition-
           broadcast when it lands).
  Phase C: out-proj outT[e, q] = sum_d w_oT[d, e] * yT[d, q].
"""

import numpy as np
import ml_dtypes

import concourse.bass as bass
import concourse.tile as tile
import concourse.mybir as mybir
from concourse import bacc
from concourse.bass_utils import run_bass_kernel_spmd

BF16 = ml_dtypes.bfloat16
F32 = mybir.dt.float32
BF = mybir.dt.bfloat16

B, L, D, H, HD = 4, 2048, 2048, 16, 128
NH = 8                      # heads per core
DL = NH * HD                # 1024 local head dims
ROPE_BASE = 10000.0
ALPHA = float(HD) ** -0.5

LT = L // 128               # 16 L-tiles
DT = D // 128               # 16 D(contract)-tiles
NCH = 6                     # qkv chunks of 512 comps: q03,k03,v03,q47,k47,v47
QC = L // 512               # 4 q-chunks of 512
KT = L // 128               # 16 k-tiles


def _chunk_kind(c):
    # chunk order: q(heads0-3), k(0-3), v(0-3), q(4-7), k(4-7), v(4-7)
    return ("q", "k", "v")[c % 3], c // 3


def build_program(phases="ABC", la=3, scb=4, ypb=2, psab=3, patb=2, paob=3):
    nc = bacc.Bacc("TRN2", target_bir_lowering=False, debug=False, num_devices=8)

    xT = nc.dram_tensor("xT", [D, L], BF, kind="ExternalInput").ap()
    wqkvT = nc.dram_tensor("wqkvT", [D, 3 * DL], BF, kind="ExternalInput").ap()
    woT = nc.dram_tensor("woT", [DL, L], BF, kind="ExternalInput").ap()
    chalf = nc.dram_tensor("chalf", [L, 256], BF, kind="ExternalInput").ap()
    shalf = nc.dram_tensor("shalf", [L, 256], BF, kind="ExternalInput").ap()
    masks = nc.dram_tensor("masks", [4 * 128, 512], BF, kind="ExternalInput").ap()
    outT = nc.dram_tensor("outT", [D, L], F32, kind="ExternalOutput").ap()

    # DRAM staging for rotated q/k (natural layout) and v
    qrot = nc.dram_tensor("qrot", [L, DL], BF, kind="Internal").ap()
    krot = nc.dram_tensor("krot", [L, DL], BF, kind="Internal").ap()
    vnat = nc.dram_tensor("vnat", [L, DL], BF, kind="Internal").ap()

    with tile.TileContext(nc) as tc:
        with tc.tile_pool(name="outer", bufs=1) as outer, \
             tc.tile_pool(name="pBqk", bufs=2, side="right") as pb, \
             tc.tile_pool(name="pBm", bufs=1, side="right") as pbm:
            # persistent: per-head attn outputs yT (rhs of phase C)
            yts = []
            for h in range(NH):
                yt = outer.tile([128, L], BF, name=f"yt{h}", tag=f"yt{h}")
                if "B" not in phases:
                    nc.vector.memset(yt, 0.0)
                yts.append(yt)
            ones128 = outer.tile([128, 128], BF, name="ones128", tag="oc")
            nc.vector.memset(ones128, 1.0)
            mts = []
            for m in range(4):
                mt = pbm.tile([128, 512], BF, name=f"mask{m}", tag=f"mask{m}")
                nc.sync.dma_start(out=mt, in_=masks[m * 128:(m + 1) * 128, :])
                mts.append(mt)

            # ---------------- Phase A: QKV + RoPE ----------------
            with tc.tile_pool(name="pA", bufs=1) as pa, \
                 tc.tile_pool(name="pAw", bufs=2) as paw, \
                 tc.tile_pool(name="pAt", bufs=patb) as pat, \
                 tc.tile_pool(name="pAo", bufs=paob) as pao, \
                 tc.tile_pool(name="psA", bufs=psab, space="PSUM") as psa:
                # resident xT tiles [128, L] per D-tile
                xts = []
                for d in range(DT):
                    xt = pa.tile([128, L], BF, name=f"xt{d}", tag=f"xt{d}")
                    nc.sync.dma_start(out=xt, in_=xT[d * 128:(d + 1) * 128, :])
                    xts.append(xt)
                c_sb = pa.tile([128, LT, 256], BF, name="c_sb", tag="c_sb")
                nc.sync.dma_start(
                    out=c_sb, in_=chalf.rearrange("(i p) g -> p i g", p=128))
                s_sb = pa.tile([128, LT, 256], BF, name="s_sb", tag="s_sb")
                nc.sync.dma_start(
                    out=s_sb, in_=shalf.rearrange("(i p) g -> p i g", p=128))

                for c in range(NCH if "A" in phases else 0):
                    kind, grp = _chunk_kind(c)
                    # weight chunk: tiles [128, 512] per D-tile
                    wch = paw.tile([128, DT, 512], BF, name="wch", tag="wch")
                    nc.sync.dma_start(
                        out=wch,
                        in_=wqkvT[:, c * 512:(c + 1) * 512].rearrange(
                            "(d p) e -> p d e", p=128))
                    for i in range(LT):
                        pnat = psa.tile([128, 512], F32, name="pnat", tag="pnat")
                        for d in range(DT):
                            nc.tensor.matmul(
                                pnat,
                                xts[d][:, i * 128:(i + 1) * 128],
                                wch[:, d, :],
                                start=(d == 0), stop=(d == DT - 1))
                        if kind == "v":
                            vo = pao.tile([128, 512], BF, name="vo", tag="ro")
                            nc.scalar.copy(out=vo, in_=pnat)
                            nc.sync.dma_start(
                                out=vnat[i * 128:(i + 1) * 128,
                                         grp * 512:(grp + 1) * 512],
                                in_=vo)
                        else:
                            x1 = pnat[:, 0::2]
                            x2 = pnat[:, 1::2]
                            ct = c_sb[:, i, :]
                            st = s_sb[:, i, :]
                            t1 = pat.tile([128, 256], F32, name="t1", tag="t1")
                            nc.vector.tensor_mul(t1, x1, ct)
                            t2 = pat.tile([128, 256], F32, name="t2", tag="t2")
                            nc.vector.tensor_mul(t2, x2, st)
                            t3 = pat.tile([128, 256], F32, name="t3", tag="t3")
                            nc.vector.tensor_mul(t3, x2, ct)
                            t4 = pat.tile([128, 256], F32, name="t4", tag="t4")
                            nc.vector.tensor_mul(t4, x1, st)
                            ro = pao.tile([128, 512], BF, name="ro", tag="ro")
                            nc.vector.tensor_sub(ro[:, 0::2], t1, t2)
                            nc.vector.tensor_add(ro[:, 1::2], t3, t4)
                            dst = qrot if kind == "q" else krot
                            nc.sync.dma_start(
                                out=dst[i * 128:(i + 1) * 128,
                                        grp * 512:(grp + 1) * 512],
                                in_=ro)

            # ---------------- Phase B: attention per head ----------------
            with tc.tile_pool(name="pBw", bufs=1) as pbw:
                # phase-C weights: loaded early in B (left side, after A frees)
                wos = []
                for dd in range(NH):
                    wo = pbw.tile([128, L], BF, name=f"wo{dd}", tag=f"wo{dd}")
                    nc.sync.dma_start(
                        out=wo, in_=woT[dd * 128:(dd + 1) * 128, :])
                    wos.append(wo)

                with tc.tile_pool(name="pBa", bufs=4) as pba, \
                     tc.tile_pool(name="pBr", bufs=2) as pbr, \
                     tc.tile_pool(name="psS", bufs=scb, space="PSUM") as pss, \
                     tc.tile_pool(name="psY", bufs=ypb, space="PSUM") as psy, \
                     tc.tile_pool(name="psD", bufs=2, space="PSUM") as psd:
                    for h in range(NH if "B" in phases else 0):
                        qt = pb.tile([128, L], BF, name="qt", tag="qt")
                        nc.sync.dma_start_transpose(
                            out=qt, in_=qrot[:, h * 128:(h + 1) * 128])
                        kt = pb.tile([128, L], BF, name="kt", tag="kt")
                        nc.sync.dma_start_transpose(
                            out=kt, in_=krot[:, h * 128:(h + 1) * 128])
                        vt = pb.tile([128, KT, 128], BF, name="vt", tag="vt")
                        nc.sync.dma_start(
                            out=vt,
                            in_=vnat[:, h * 128:(h + 1) * 128].rearrange(
                                "(j p) d -> p j d", p=128))

                        for qc in range(QC):
                            nkt = 4 * qc + 4
                            ypsum = psy.tile([128, 512], F32, name="ypsum", tag="yp")
                            dpsum = psd.tile([128, 512], F32, name="dpsum", tag="dp")
                            ats = {}

                            def emit_score(j, qc=qc, ats=ats):
                                sc = pss.tile([128, 512], F32, name="sc", tag="sc")
                                nc.tensor.matmul(
                                    sc, kt[:, j * 128:(j + 1) * 128],
                                    qt[:, qc * 512:(qc + 1) * 512],
                                    start=True, stop=True)
                                at = pba.tile([128, 512], BF, name="at", tag="at")
                                nc.scalar.activation(
                                    out=at, in_=sc,
                                    func=mybir.ActivationFunctionType.Exp,
                                    scale=ALPHA)
                                m = j - 4 * qc
                                if m >= 0:
                                    nc.vector.tensor_mul(at, at, mts[m])
                                ats[j] = at

                            LOOKAHEAD = la
                            for j in range(min(LOOKAHEAD, nkt)):
                                emit_score(j)
                            for j in range(nkt):
                                if j + LOOKAHEAD < nkt:
                                    emit_score(j + LOOKAHEAD)
                                at = ats.pop(j)
                                nc.tensor.matmul(
                                    ypsum, vt[:, j, :], at,
                                    start=(j == 0), stop=(j == nkt - 1))
                                nc.tensor.matmul(
                                    dpsum, ones128, at,
                                    start=(j == 0), stop=(j == nkt - 1))
                            rbs = pbr.tile([128, 512], BF, name="rbs", tag="rbs")
                            with nc.allow_low_precision("softmax recip bf16"):
                                nc.vector.reciprocal(out=rbs, in_=dpsum)
                            nc.vector.tensor_mul(
                                yts[h][:, qc * 512:(qc + 1) * 512], ypsum, rbs)

                # ---------------- Phase C: out-projection ----------------
                with tc.tile_pool(name="pCo", bufs=4) as pco, \
                     tc.tile_pool(name="psC", bufs=3, space="PSUM") as psc:
                    for e in range(DT if "C" in phases else 0):
                        for qc in range(QC):
                            op = psc.tile([128, 512], F32, name="op", tag="op")
                            for dd in range(NH):
                                nc.tensor.matmul(
                                    op,
                                    wos[dd][:, e * 128:(e + 1) * 128],
                                    yts[dd][:, qc * 512:(qc + 1) * 512],
                                    start=(dd == 0), stop=(dd == NH - 1))
                            ot = pco.tile([128, 512], F32, name="ot", tag="ot")
                            nc.scalar.copy(out=ot, in_=op)
                            nc.sync.dma_start(
                                out=outT[e * 128:(e + 1) * 128,
                                         qc * 512:(qc + 1) * 512],
                                in_=ot)
    nc.compile()
    return nc


_NC_CACHE = None


def _get_program():
    global _NC_CACHE
    if _NC_CACHE is None:
        _NC_CACHE = build_program()
    return _NC_CACHE


def _host_inputs(x, w_qkv, w_o):
    inv = 1.0 / (ROPE_BASE ** (np.arange(0, HD, 2, dtype=np.float64) / HD))
    ang = np.arange(L, dtype=np.float64)[:, None] * inv[None, :]
    chalf = np.tile(np.cos(ang), (1, 4)).astype(BF16)          # [L, 256]
    shalf = np.tile(np.sin(ang), (1, 4)).astype(BF16)
    p = np.arange(128)[:, None]
    f = np.arange(512)[None, :]
    masks = np.concatenate(
        [(128 * m + p <= f).astype(BF16) for m in range(4)], axis=0)  # [512,512]

    in_maps = []
    for c in range(8):
        b, g = c % 4, c // 4
        qr = w_qkv[g * DL:(g + 1) * DL]
        kr = w_qkv[D + g * DL:D + (g + 1) * DL]
        vr = w_qkv[2 * D + g * DL:2 * D + (g + 1) * DL]
        wqkvT = np.ascontiguousarray(
            np.concatenate([qr[:512], kr[:512], vr[:512],
                            qr[512:], kr[512:], vr[512:]], axis=0).T
        ).astype(BF16)
        in_maps.append({
            "xT": np.ascontiguousarray(x[b].T).astype(BF16),
            "wqkvT": wqkvT,
            "woT": np.ascontiguousarray(
                w_o[:, g * DL:(g + 1) * DL].T).astype(BF16),
            "chalf": chalf,
            "shalf": shalf,
            "masks": masks,
        })
    return in_maps


def kernel(x, w_qkv, w_o, _trace=False):
    x = np.asarray(x, dtype=np.float32)
    w_qkv = np.asarray(w_qkv, dtype=np.float32)
    w_o = np.asarray(w_o, dtype=np.float32)
    nc = _get_program()
    in_maps = _host_inputs(x, w_qkv, w_o)
    res = run_bass_kernel_spmd(nc, in_maps, core_ids=list(range(8)),
                               trace=_trace)
    kernel.last_result = res
    parts = [r["outT"] for r in res.results]
    out = np.empty((B, L, D), dtype=np.float32)
    for b in range(B):
        out[b] = (parts[b] + parts[b + 4]).T
    return out



# revision 2
# speedup vs baseline: 1.0552x; 1.0552x over previous
"""MHA (RoPE + causal softmax attention + out-proj) on 8 NeuronCores — v3.

Sharding: DP4 x TP2 (core c: batch c % 4, head-group c // 4; 8 heads/core).
Host sums the two head-group partial outputs per batch and transposes.

Key structure (tuned against the TimelineSim cost model):
  * Phase A (QKV) matmuls run in fp8 e4m3 DoubleRow with a hi/lo split of
    both operands (x ~ xh + xl/16, 64w ~ wh + wl/16, lo*lo dropped):
    3 DR instructions per d-tile pair = 0.75x the bf16 instruction cost at
    better-than-bf16 accuracy. PSUM carries 1024*qkv; the 2^-10 unscale is
    folded into the RoPE-cast / v copy scales.
  * q/k comps are host-permuted to (evens | odds) within each head so RoPE
    reads contiguous PSUM blocks: 2 ACT casts + 6 DVE bf16 ops (4x mode).
  * Softmax: exp(alpha*s - 8ln2) -> fp16 exp tiles; denominator accumulated
    with DVE adds + ONE ones-matmul per (head, qc) instead of a ones-matmul
    per k-tile. Causal diagonal tiles are column-trimmed; a single [128,128]
    triangle mask remains.
  * v never round-trips DRAM (PSUM -> SBUF fp16 copy, resident).
  * Emission interleaves head-group-1 QKV tiles into the attention loop of
    head-group-0 so the exp-bound stretch of attention overlaps the
    PE-bound QKV GEMM instead of stalling the tensor engine.
"""

import numpy as np
import ml_dtypes

import concourse.bass as bass
import concourse.tile as tile
import concourse.mybir as mybir
from concourse import bacc
from concourse.bass_utils import run_bass_kernel_spmd

BF16 = ml_dtypes.bfloat16
F8NP = ml_dtypes.float8_e4m3
F32 = mybir.dt.float32
BF = mybir.dt.bfloat16
F16 = mybir.dt.float16
F8 = mybir.dt.float8e4
DR = mybir.MatmulPerfMode.DoubleRow

B, L, D, H, HD = 4, 2048, 2048, 16, 128
NH = 8                      # heads per core
DL = NH * HD                # 1024 local head dims
ROPE_BASE = 10000.0
ALPHA = float(HD) ** -0.5
EXP_BIAS = -8.0 * float(np.log(2.0))   # exp(a*s - 8ln2): keeps fp16 sums safe

LT = L // 128               # 16 L-tiles
DT = D // 128               # 16 D(contract)-tiles
NCH = 6                     # qkv chunks of 512 comps: q03,k03,v03,q47,k47,v47
QC = L // 512               # 4 q-chunks of 512
KT = L // 128               # 16 k-tiles


def _chunk_kind(c):
    # chunk order: q(heads0-3), k(0-3), v(0-3), q(4-7), k(4-7), v(4-7)
    return ("q", "k", "v")[c % 3], c // 3


def build_program(phases="ABC", la=2, scb=3, ypb=1, psab=3, patb=1, paob=2,
                  pbab=6, take=3):
    nc = bacc.Bacc("TRN2", target_bir_lowering=False, debug=False, num_devices=8)

    # x hi/lo fp8 planes: hi = fp8(x), lo = fp8(16*(x-hi))
    xThi = nc.dram_tensor("xThi", [D, L], F8, kind="ExternalInput").ap()
    xTlo = nc.dram_tensor("xTlo", [D, L], F8, kind="ExternalInput").ap()
    # w planes: wA = fp8(16*w64_hi) [D, 3DL]; wB [D, 2*3DL]: per 512-chunk,
    # 1024 cols = (w64_hi 512 | w16_lo 512)
    wA = nc.dram_tensor("wA", [D, 3 * DL], F8, kind="ExternalInput").ap()
    wB = nc.dram_tensor("wB", [D, 6 * DL], F8, kind="ExternalInput").ap()
    woT = nc.dram_tensor("woT", [DL, L], BF, kind="ExternalInput").ap()
    chalf = nc.dram_tensor("chalf", [L, 256], BF, kind="ExternalInput").ap()
    shalf = nc.dram_tensor("shalf", [L, 256], BF, kind="ExternalInput").ap()
    tri = nc.dram_tensor("tri", [128, 128], F16, kind="ExternalInput").ap()
    outT = nc.dram_tensor("outT", [D, L], BF, kind="ExternalOutput").ap()

    # DRAM staging for rotated q/k, split per head-group so group-0 attention
    # does not depend on group-1 writes
    qkrot = [[nc.dram_tensor(f"{nm}rot{g}", [L, 512], BF, kind="Internal").ap()
              for g in range(2)] for nm in ("q", "k")]

    doA = "A" in phases
    doB = "B" in phases
    doC = "C" in phases

    with tile.TileContext(nc) as tc:
        outer_cm = tc.tile_pool(name="outer", bufs=1)
        pb_cm = tc.tile_pool(name="pBqk", bufs=2, side="right")
        pbm_cm = tc.tile_pool(name="pBm", bufs=1, side="right")
        pby03_cm = tc.tile_pool(name="pBy03", bufs=1)
        pba_cm = tc.tile_pool(name="pBa", bufs=pbab)
        pbr_cm = tc.tile_pool(name="pBr", bufs=2)
        pbd_cm = tc.tile_pool(name="pBd", bufs=2)
        pss_cm = tc.tile_pool(name="psS", bufs=scb, space="PSUM")
        psy_cm = tc.tile_pool(name="psY", bufs=ypb, space="PSUM")
        psd_cm = tc.tile_pool(name="psD", bufs=1, space="PSUM")
        with outer_cm as outer, pb_cm as pb, pbm_cm as pbm, \
             pby03_cm as pby03, pba_cm as pba, pbr_cm as pbr, \
             pbd_cm as pbd, pss_cm as pss, psy_cm as psy, psd_cm as psd:
            # v for both head groups, resident SBUF: [128(kpos), LT, 512]
            vsb = [outer.tile([128, LT, 512], F16, name=f"vsb{g}",
                              tag=f"vsb{g}") for g in range(2)]
            ones128 = outer.tile([128, 128], F16, name="ones128", tag="oc")
            nc.vector.memset(ones128, 1.0)
            ebias = outer.tile([128, 1], F32, name="ebias", tag="ebias")
            nc.vector.memset(ebias, EXP_BIAS)
            trit = pbm.tile([128, 128], F16, name="tri", tag="tri")
            nc.sync.dma_start(out=trit, in_=tri)

            yts = {}
            for h in range(4):
                yts[h] = pby03.tile([128, L], BF, name=f"yt{h}", tag=f"yt{h}")
                if not doB:
                    nc.vector.memset(yts[h], 0.0)

            # ---------------- phase B helpers ----------------
            def load_qk(h):
                grp, hh = h // 4, h % 4
                qt = pb.tile([128, L], BF, name="qt", tag="qt")
                kt = pb.tile([128, L], BF, name="kt", tag="kt")
                for t, src in ((qt, qkrot[0][grp]), (kt, qkrot[1][grp])):
                    nc.sync.dma_start_transpose(
                        out=t, in_=src[:, hh * 128:(hh + 1) * 128])
                return qt, kt

            def emit_B_qc(h, qt, kt, qc):
                grp, hh = h // 4, h % 4
                nkt = 4 * qc + 4
                ypsum = psy.tile([128, 512], F32, name="ypsum", tag="yp")
                dacc = pbd.tile([128, 512], F16, name="dacc", tag="dacc")
                ats = {}

                def emit_score(j):
                    m = j - 4 * qc
                    off = 128 * m if m > 0 else 0
                    w = 512 - off
                    sc = pss.tile([128, 512], F32, name="sc", tag="sc")
                    nc.tensor.matmul(
                        sc[:, 0:w], kt[:, j * 128:(j + 1) * 128],
                        qt[:, qc * 512 + off:(qc + 1) * 512],
                        start=True, stop=True)
                    at = pba.tile([128, 512], F16, name="at", tag="at")
                    nc.scalar.activation(
                        out=at[:, 0:w], in_=sc[:, 0:w],
                        func=mybir.ActivationFunctionType.Exp,
                        scale=ALPHA, bias=ebias)
                    if m >= 0:
                        nc.vector.tensor_mul(at[:, 0:128], at[:, 0:128], trit)
                    ats[j] = (at, off, w)

                for j in range(min(la, nkt)):
                    emit_score(j)
                prev = None
                for j in range(nkt):
                    if j + la < nkt:
                        emit_score(j + la)
                    at, off, w = ats.pop(j)
                    nc.tensor.matmul(
                        ypsum[:, off:512],
                        vsb[grp][:, j, hh * 128:(hh + 1) * 128],
                        at[:, 0:w],
                        start=(j == 0), stop=(j == nkt - 1),
                        skip_group_check=True)
                    if j == 0:
                        prev = at
                    elif j == 1:
                        if qc == 0:
                            nc.vector.tensor_copy(
                                out=dacc[:, 0:128], in_=prev[:, 0:128])
                            nc.vector.tensor_add(
                                dacc[:, 128:512], prev[:, 128:512],
                                at[:, 0:w])
                        else:
                            nc.vector.tensor_add(dacc, prev, at)
                    else:
                        nc.vector.tensor_add(
                            dacc[:, off:512], dacc[:, off:512], at[:, 0:w])
                dpsum = psd.tile([128, 512], F32, name="dpsum", tag="dp")
                nc.tensor.matmul(dpsum, ones128, dacc, start=True, stop=True)
                rbs = pbr.tile([128, 512], BF, name="rbs", tag="rbs")
                with nc.allow_low_precision("softmax recip bf16"):
                    nc.vector.reciprocal(out=rbs, in_=dpsum)
                nc.vector.tensor_mul(
                    yts[h][:, qc * 512:(qc + 1) * 512], ypsum, rbs)

            # ---------------- phase A scope + interleave ----------------
            with tc.tile_pool(name="pA", bufs=1) as pa, \
                 tc.tile_pool(name="pAw", bufs=2) as paw, \
                 tc.tile_pool(name="pAt", bufs=patb) as pat, \
                 tc.tile_pool(name="pAo", bufs=paob) as pao, \
                 tc.tile_pool(name="psA", bufs=psab, space="PSUM") as psa:
                xall = pa.tile([128, DT, 2, L], F8, name="xall", tag="xall")
                c_sb = pa.tile([128, LT, 256], BF, name="c_sb", tag="c_sb")
                s_sb = pa.tile([128, LT, 256], BF, name="s_sb", tag="s_sb")
                wch = {}

                def load_wch(c):
                    if c >= NCH or c in wch:
                        return
                    wa = paw.tile([128, DT, 512], F8, name="wchA", tag="wchA")
                    wb = paw.tile([128, DT, 2, 512], F8, name="wchB",
                                  tag="wchB")
                    wAr = wA[:, c * 512:(c + 1) * 512].rearrange(
                        "(d p) e -> p d e", p=128)
                    wBr = wB[:, c * 1024:(c + 1) * 1024].rearrange(
                        "(d p) e -> p d e", p=128)
                    wbf = wb.rearrange("p d t e -> p d (t e)")
                    for d2 in range(DT // 2):
                        sl = slice(2 * d2, 2 * d2 + 2)
                        nc.sync.dma_start(out=wa[:, sl, :], in_=wAr[:, sl, :])
                        nc.sync.dma_start(out=wbf[:, sl, :], in_=wBr[:, sl, :])
                    wch[c] = (wa, wb)

                def load_x():
                    wa = paw.tile([128, DT, 512], F8, name="wchA", tag="wchA")
                    wb = paw.tile([128, DT, 2, 512], F8, name="wchB",
                                  tag="wchB")
                    wAr = wA[:, 0:512].rearrange("(d p) e -> p d e", p=128)
                    wBr = wB[:, 0:1024].rearrange("(d p) e -> p d e", p=128)
                    wbf = wb.rearrange("p d t e -> p d (t e)")
                    for d in range(DT):
                        nc.sync.dma_start(
                            out=xall[:, d, 1, :],
                            in_=xThi[d * 128:(d + 1) * 128, :])
                        nc.sync.dma_start(
                            out=xall[:, d, 0, :],
                            in_=xTlo[d * 128:(d + 1) * 128, :])
                        if d % 2 == 1:
                            sl = slice(d - 1, d + 1)
                            nc.sync.dma_start(out=wa[:, sl, :],
                                              in_=wAr[:, sl, :])
                            nc.sync.dma_start(out=wbf[:, sl, :],
                                              in_=wBr[:, sl, :])
                            i2 = d // 2
                            for t_sb, t_dr in ((c_sb, chalf), (s_sb, shalf)):
                                for i in (2 * i2, 2 * i2 + 1):
                                    nc.sync.dma_start(
                                        out=t_sb[:, i, :],
                                        in_=t_dr[i * 128:(i + 1) * 128, :])
                    wch[0] = (wa, wb)

                def emit_A_alpha(c, i):
                    wa, _ = wch[c]
                    ls = slice(i * 128, (i + 1) * 128)
                    pnat = psa.tile([128, 512], F32, name="pnat", tag="pnat")
                    for d2 in range(DT // 2):
                        nc.tensor.matmul(
                            pnat,
                            xall[:, 2 * d2:2 * d2 + 2, 1, ls],
                            wa[:, 2 * d2:2 * d2 + 2, :],
                            start=(d2 == 0), stop=False, perf_mode=DR)
                    return pnat

                def emit_A_finish(c, i, pnat):
                    kind, grp = _chunk_kind(c)
                    _, wb = wch[c]
                    if i == 8:
                        load_wch(c + 1)
                    ls = slice(i * 128, (i + 1) * 128)
                    for d in range(DT):
                        nc.tensor.matmul(
                            pnat,
                            xall[:, d, :, ls],
                            wb[:, d, :, :],
                            start=False, stop=(d == DT - 1), perf_mode=DR)
                    if kind == "v":
                        nc.scalar.activation(
                            out=vsb[grp][:, i, :], in_=pnat,
                            func=mybir.ActivationFunctionType.Copy,
                            scale=1.0 / 1024.0)
                        return
                    # RoPE: per-head comps are permuted (evens | odds)
                    pv = pnat.rearrange("p (hh t z) -> p hh t z",
                                        hh=4, t=2, z=64)
                    x1 = pat.tile([128, 256], BF, name="x1", tag="x1")
                    nc.scalar.activation(
                        out=x1, in_=pv[:, :, 0, :],
                        func=mybir.ActivationFunctionType.Copy,
                        scale=1.0 / 1024.0)
                    x2 = pat.tile([128, 256], BF, name="x2", tag="x2")
                    nc.scalar.activation(
                        out=x2, in_=pv[:, :, 1, :],
                        func=mybir.ActivationFunctionType.Copy,
                        scale=1.0 / 1024.0)
                    ct = c_sb[:, i, :]
                    st = s_sb[:, i, :]
                    t1 = pat.tile([128, 256], BF, name="t1", tag="t1")
                    nc.vector.tensor_mul(t1, x1, ct)
                    t2 = pat.tile([128, 256], BF, name="t2", tag="t2")
                    nc.vector.tensor_mul(t2, x2, st)
                    t3 = pat.tile([128, 256], BF, name="t3", tag="t3")
                    nc.vector.tensor_mul(t3, x2, ct)
                    t4 = pat.tile([128, 256], BF, name="t4", tag="t4")
                    nc.vector.tensor_mul(t4, x1, st)
                    ro = pao.tile([128, 512], BF, name="ro", tag="ro")
                    rv = ro.rearrange("p (hh t z) -> p hh t z", hh=4, t=2, z=64)
                    nc.vector.tensor_sub(rv[:, :, 0, :], t1, t2)
                    nc.vector.tensor_add(rv[:, :, 1, :], t3, t4)
                    dst = qkrot[0 if kind == "q" else 1][grp]
                    nc.sync.dma_start(out=dst[ls, :], in_=ro)

                def emit_A_tile(c, i):
                    emit_A_finish(c, i, emit_A_alpha(c, i))

                # ---- emission: A(g0), then heads 0-3 x A(g1) ----
                if doA:
                    load_x()
                    # chunk 0: software-pipeline alpha/beta so the alpha
                    # chain starts as soon as the hi plane + wA arrive
                    pns = {}
                    pns[0] = emit_A_alpha(0, 0)
                    pns[1] = emit_A_alpha(0, 1)
                    for i in range(LT):
                        emit_A_finish(0, i, pns.pop(i))
                        if i + 2 < LT:
                            pns[i + 2] = emit_A_alpha(0, i + 2)
                    for c in (1, 2):
                        for i in range(LT):
                            emit_A_tile(c, i)
                g1 = [(c, i) for c in (3, 4, 5) for i in range(LT)] \
                    if doA else []
                gi = 0
                qts = {}
                for h in range(4 if doB else 0):
                    if h not in qts:
                        qts[h] = load_qk(h)
                    qt, kt = qts.pop(h)
                    for qc in range(QC):
                        emit_B_qc(h, qt, kt, qc)
                        if qc == 0 and h < 4 and doB:
                            qts[h + 1] = load_qk(h + 1)
                        for _ in range(take):
                            if gi < len(g1):
                                emit_A_tile(*g1[gi])
                                gi += 1
                while gi < len(g1):
                    emit_A_tile(*g1[gi])
                    gi += 1

            # ---------------- heads 4-7 (qc-major) + phase C ----------
            with tc.tile_pool(name="pBy47", bufs=1) as pby47, \
                 tc.tile_pool(name="pCo", bufs=4) as pco, \
                 tc.tile_pool(name="psC", bufs=2, space="PSUM") as psc:
                qk47 = dict(qts)   # h4 was prefetched into the pb pool
                for h in range(5, NH if doB else 5):
                    grp, hh = h // 4, h % 4
                    qt = pby47.tile([128, L], BF, name=f"qt{h}", tag=f"qt{h}")
                    kt = pby47.tile([128, L], BF, name=f"kt{h}", tag=f"kt{h}")
                    for t, src in ((qt, qkrot[0][grp]), (kt, qkrot[1][grp])):
                        nc.sync.dma_start_transpose(
                            out=t, in_=src[:, hh * 128:(hh + 1) * 128])
                    qk47[h] = (qt, kt)
                for h in range(4, NH):
                    yts[h] = pby47.tile([128, L], BF, name=f"yt{h}",
                                        tag=f"yt{h}")
                    if not doB:
                        nc.vector.memset(yts[h], 0.0)
                wos = []
                for dd in range(NH):
                    wo = pby47.tile([128, L], BF, name=f"wo{dd}", tag=f"wo{dd}")
                    nc.sync.dma_start(
                        out=wo, in_=woT[dd * 128:(dd + 1) * 128, :])
                    wos.append(wo)

                def emit_C(e, qc):
                    op = psc.tile([128, 512], F32, name="op", tag="op")
                    for dd in range(NH):
                        nc.tensor.matmul(
                            op,
                            wos[dd][:, e * 128:(e + 1) * 128],
                            yts[dd][:, qc * 512:(qc + 1) * 512],
                            start=(dd == 0), stop=(dd == NH - 1))
                    ot = pco.tile([128, 512], BF, name="ot", tag="ot")
                    nc.vector.tensor_copy(out=ot, in_=op)
                    nc.sync.dma_start(
                        out=outT[e * 128:(e + 1) * 128,
                                 qc * 512:(qc + 1) * 512],
                        in_=ot)

                for qc in range(QC if doB else 0):
                    for h in range(4, NH):
                        emit_B_qc(h, qk47[h][0], qk47[h][1], qc)
                    if doC and qc > 0:
                        for e in range(DT):
                            emit_C(e, qc - 1)
                if doC:
                    for qc in ([3] if doB else range(QC)):
                        for e in range(DT):
                            emit_C(e, qc)
    nc.compile()
    return nc


_NC_CACHE = None


def _get_program():
    global _NC_CACHE
    if _NC_CACHE is None:
        _NC_CACHE = build_program()
    return _NC_CACHE


def _f8(a):
    return np.clip(np.asarray(a, np.float64), -240.0, 240.0).astype(F8NP)


# within each head's 128 comps: evens first, then odds
_PERM512 = np.concatenate(
    [np.concatenate([np.arange(h * 128, (h + 1) * 128, 2),
                     np.arange(h * 128 + 1, (h + 1) * 128, 2)])
     for h in range(4)])


def _host_inputs(x, w_qkv, w_o):
    inv = 1.0 / (ROPE_BASE ** (np.arange(0, HD, 2, dtype=np.float64) / HD))
    ang = np.arange(L, dtype=np.float64)[:, None] * inv[None, :]
    chalf = np.tile(np.cos(ang), (1, 4)).astype(BF16)          # [L, 256]
    shalf = np.tile(np.sin(ang), (1, 4)).astype(BF16)
    p = np.arange(128)[:, None]
    f = np.arange(128)[None, :]
    tri = (p <= f).astype(np.float16)                          # [128, 128]

    xhi_all, xlo_all = [], []
    for b in range(B):
        xb = np.asarray(x[b], np.float64)                      # [L, D]
        xh = _f8(xb)
        xl = _f8(16.0 * (xb - xh.astype(np.float64)))
        xhi_all.append(np.ascontiguousarray(xh.T))
        xlo_all.append(np.ascontiguousarray(xl.T))

    in_maps = []
    for c in range(8):
        b, g = c % 4, c // 4
        qr = w_qkv[g * DL:(g + 1) * DL]
        kr = w_qkv[D + g * DL:D + (g + 1) * DL]
        vr = w_qkv[2 * D + g * DL:2 * D + (g + 1) * DL]
        qp = qr[np.concatenate([_PERM512, 512 + _PERM512])]
        kp = kr[np.concatenate([_PERM512, 512 + _PERM512])]
        wloc = np.concatenate([qp[:512], kp[:512], vr[:512],
                               qp[512:], kp[512:], vr[512:]],
                              axis=0).astype(np.float64)       # [3DL, D]
        w64h = _f8(64.0 * wloc)
        w1024h = (w64h.astype(np.float64) * 16.0).astype(F8NP)  # exact shift
        w16l = _f8(16.0 * (64.0 * wloc - w64h.astype(np.float64)))
        wAh = np.ascontiguousarray(w1024h.T)                    # [D, 3DL]
        w64hT, w16lT = w64h.T, w16l.T                           # [D, 3DL]
        wBh = np.ascontiguousarray(np.concatenate(
            [np.stack([w64hT[:, c2 * 512:(c2 + 1) * 512],
                       w16lT[:, c2 * 512:(c2 + 1) * 512]],
                      axis=1).reshape(D, 1024)
             for c2 in range(NCH)], axis=1))                    # [D, 6DL]
        in_maps.append({
            "xThi": xhi_all[b],
            "xTlo": xlo_all[b],
            "wA": wAh,
            "wB": wBh,
            "woT": np.ascontiguousarray(
                w_o[:, g * DL:(g + 1) * DL].T).astype(BF16),
            "chalf": chalf,
            "shalf": shalf,
            "tri": tri,
        })
    return in_maps


def kernel(x, w_qkv, w_o, _trace=False):
    x = np.asarray(x, dtype=np.float32)
    w_qkv = np.asarray(w_qkv, dtype=np.float32)
    w_o = np.asarray(w_o, dtype=np.float32)
    nc = _get_program()
    in_maps = _host_inputs(x, w_qkv, w_o)
    res = run_bass_kernel_spmd(nc, in_maps, core_ids=list(range(8)),
                               trace=_trace)
    kernel.last_result = res
    parts = [r["outT"].astype(np.float32) for r in res.results]
    out = np.empty((B, L, D), dtype=np.float32)
    for b in range(B):
        out[b] = (parts[b] + parts[b + 4]).T
    return out


# revision 3
# speedup vs baseline: 1.0797x; 1.0232x over previous
"""MHA (RoPE + causal softmax attention + out-proj) on 8 NeuronCores — v3.

Sharding: DP4 x TP2 (core c: batch c % 4, head-group c // 4; 8 heads/core).
Host sums the two head-group partial outputs per batch and transposes.

Key structure (tuned against the TimelineSim cost model):
  * Phase A (QKV) matmuls run in fp8 e4m3 DoubleRow with a hi/lo split of
    both operands (x ~ xh + xl/16, 64w ~ wh + wl/16, lo*lo dropped):
    3 DR instructions per d-tile pair = 0.75x the bf16 instruction cost at
    better-than-bf16 accuracy. PSUM carries 1024*qkv; the 2^-10 unscale is
    folded into the RoPE-cast / v copy scales.
  * q/k comps are host-permuted to (evens | odds) within each head so RoPE
    reads contiguous PSUM blocks: 2 ACT casts + 6 DVE bf16 ops (4x mode).
  * Softmax: exp(alpha*s - 8ln2) -> fp16 exp tiles; denominator accumulated
    with DVE adds + ONE ones-matmul per (head, qc) instead of a ones-matmul
    per k-tile. Causal diagonal tiles are column-trimmed; a single [128,128]
    triangle mask remains.
  * v never round-trips DRAM (PSUM -> SBUF fp16 copy, resident).
  * Emission interleaves head-group-1 QKV tiles into the attention loop of
    head-group-0 so the exp-bound stretch of attention overlaps the
    PE-bound QKV GEMM instead of stalling the tensor engine.
"""

import numpy as np
import ml_dtypes

import concourse.bass as bass
import concourse.tile as tile
import concourse.mybir as mybir
from concourse import bacc
from concourse.bass_utils import run_bass_kernel_spmd

BF16 = ml_dtypes.bfloat16
F8NP = ml_dtypes.float8_e4m3
F32 = mybir.dt.float32
BF = mybir.dt.bfloat16
F16 = mybir.dt.float16
F8 = mybir.dt.float8e4
DR = mybir.MatmulPerfMode.DoubleRow

B, L, D, H, HD = 4, 2048, 2048, 16, 128
NH = 8                      # heads per core
DL = NH * HD                # 1024 local head dims
ROPE_BASE = 10000.0
ALPHA = float(HD) ** -0.5
EXP_BIAS = -8.0 * float(np.log(2.0))   # exp(a*s - 8ln2): keeps fp16 sums safe

LT = L // 128               # 16 L-tiles
DT = D // 128               # 16 D(contract)-tiles
NCH = 6                     # qkv chunks of 512 comps: q03,k03,v03,q47,k47,v47
QC = L // 512               # 4 q-chunks of 512
KT = L // 128               # 16 k-tiles


def _chunk_kind(c):
    # chunk order: q(heads0-3), k(0-3), v(0-3), q(4-7), k(4-7), v(4-7)
    return ("q", "k", "v")[c % 3], c // 3


def build_program(phases="ABC", la=3, scb=3, ypb=1, psab=3, patb=1, paob=2,
                  pbab=6, take=3):
    nc = bacc.Bacc("TRN2", target_bir_lowering=False, debug=False, num_devices=8)

    # x hi/lo fp8 planes: hi = fp8(x), lo = fp8(16*(x-hi))
    xThi = nc.dram_tensor("xThi", [D, L], F8, kind="ExternalInput").ap()
    xTlo = nc.dram_tensor("xTlo", [D, L], F8, kind="ExternalInput").ap()
    # w planes: wA = fp8(16*w64_hi) [D, 3DL]; wB [D, 2*3DL]: per 512-chunk,
    # 1024 cols = (w64_hi 512 | w16_lo 512)
    wA = nc.dram_tensor("wA", [D, 3 * DL], F8, kind="ExternalInput").ap()
    wB = nc.dram_tensor("wB", [D, 6 * DL], F8, kind="ExternalInput").ap()
    woT = nc.dram_tensor("woT", [DL, L], BF, kind="ExternalInput").ap()
    chalf = nc.dram_tensor("chalf", [L, 256], BF, kind="ExternalInput").ap()
    shalf = nc.dram_tensor("shalf", [L, 256], BF, kind="ExternalInput").ap()
    tri = nc.dram_tensor("tri", [128, 128], F16, kind="ExternalInput").ap()
    outT = nc.dram_tensor("outT", [D, L], BF, kind="ExternalOutput").ap()

    # DRAM staging for rotated q/k, split per head-group so group-0 attention
    # does not depend on group-1 writes
    qkrot = [[nc.dram_tensor(f"{nm}rot{g}", [L, 512], BF, kind="Internal").ap()
              for g in range(2)] for nm in ("q", "k")]

    doA = "A" in phases
    doB = "B" in phases
    doC = "C" in phases

    with tile.TileContext(nc) as tc:
        outer_cm = tc.tile_pool(name="outer", bufs=1)
        pb_cm = tc.tile_pool(name="pBqk", bufs=2, side="right")
        pbm_cm = tc.tile_pool(name="pBm", bufs=1, side="right")
        pby03_cm = tc.tile_pool(name="pBy03", bufs=1)
        pba_cm = tc.tile_pool(name="pBa", bufs=pbab)
        pbr_cm = tc.tile_pool(name="pBr", bufs=2)
        pbd_cm = tc.tile_pool(name="pBd", bufs=2)
        P = {}
        with outer_cm as outer, pb_cm as pb, pbm_cm as pbm, \
             pby03_cm as pby03, pba_cm as pba, pbr_cm as pbr, \
             pbd_cm as pbd:
            # v for both head groups, resident SBUF: [128(kpos), LT, 512]
            vsb = [outer.tile([128, LT, 512], F16, name=f"vsb{g}",
                              tag=f"vsb{g}") for g in range(2)]
            ones128 = outer.tile([128, 128], F16, name="ones128", tag="oc")
            nc.vector.memset(ones128, 1.0)
            ebias = outer.tile([128, 1], F32, name="ebias", tag="ebias")
            nc.vector.memset(ebias, EXP_BIAS)
            trit = pbm.tile([128, 128], F16, name="tri", tag="tri")
            nc.sync.dma_start(out=trit, in_=tri)

            qts = {}
            yts = {}
            for h in range(4):
                yts[h] = pby03.tile([128, L], BF, name=f"yt{h}", tag=f"yt{h}")
                if not doB:
                    nc.vector.memset(yts[h], 0.0)

            # ---------------- phase B helpers ----------------
            def load_qk(h):
                grp, hh = h // 4, h % 4
                qt = pb.tile([128, L], BF, name="qt", tag="qt")
                kt = pb.tile([128, L], BF, name="kt", tag="kt")
                for t, src in ((qt, qkrot[0][grp]), (kt, qkrot[1][grp])):
                    nc.sync.dma_start_transpose(
                        out=t, in_=src[:, hh * 128:(hh + 1) * 128])
                return qt, kt

            def emit_B_qc(h, qt, kt, qc):
                grp, hh = h // 4, h % 4
                nkt = 4 * qc + 4
                ypsum = P["psy"].tile([128, 512], F32, name="ypsum", tag="yp")
                dacc = pbd.tile([128, 512], F16, name="dacc", tag="dacc")
                ats = {}

                def emit_score(j):
                    m = j - 4 * qc
                    off = 128 * m if m > 0 else 0
                    w = 512 - off
                    sc = P["pss"].tile([128, 512], F32, name="sc", tag="sc")
                    nc.tensor.matmul(
                        sc[:, 0:w], kt[:, j * 128:(j + 1) * 128],
                        qt[:, qc * 512 + off:(qc + 1) * 512],
                        start=True, stop=True)
                    at = pba.tile([128, 512], F16, name="at", tag="at")
                    nc.scalar.activation(
                        out=at[:, 0:w], in_=sc[:, 0:w],
                        func=mybir.ActivationFunctionType.Exp,
                        scale=ALPHA, bias=ebias)
                    if m >= 0:
                        nc.vector.tensor_mul(at[:, 0:128], at[:, 0:128], trit)
                    ats[j] = (at, off, w)

                for j in range(min(la, nkt)):
                    emit_score(j)
                prev = None
                for j in range(nkt):
                    if j + la < nkt:
                        emit_score(j + la)
                    at, off, w = ats.pop(j)
                    nc.tensor.matmul(
                        ypsum[:, off:512],
                        vsb[grp][:, j, hh * 128:(hh + 1) * 128],
                        at[:, 0:w],
                        start=(j == 0), stop=(j == nkt - 1),
                        skip_group_check=True)
                    if j == 0:
                        prev = at
                    elif j == 1:
                        if qc == 0:
                            nc.vector.tensor_copy(
                                out=dacc[:, 0:128], in_=prev[:, 0:128])
                            nc.vector.tensor_add(
                                dacc[:, 128:512], prev[:, 128:512],
                                at[:, 0:w])
                        else:
                            nc.vector.tensor_add(dacc, prev, at)
                    else:
                        nc.vector.tensor_add(
                            dacc[:, off:512], dacc[:, off:512], at[:, 0:w])
                dpsum = P["psd"].tile([128, 512], F32, name="dpsum", tag="dp")
                nc.tensor.matmul(dpsum, ones128, dacc, start=True, stop=True)
                rbs = pbr.tile([128, 512], BF, name="rbs", tag="rbs")
                with nc.allow_low_precision("softmax recip bf16"):
                    nc.vector.reciprocal(out=rbs, in_=dpsum)
                nc.vector.tensor_mul(
                    yts[h][:, qc * 512:(qc + 1) * 512], ypsum, rbs)

            # ---------------- phase A scope + interleave ----------------
            with tc.tile_pool(name="pA", bufs=1) as pa, \
                 tc.tile_pool(name="pAw", bufs=2) as paw, \
                 tc.tile_pool(name="pAt", bufs=patb) as pat, \
                 tc.tile_pool(name="pAo", bufs=paob) as pao:
                xall = pa.tile([128, DT, 2, L], F8, name="xall", tag="xall")
                c_sb = pa.tile([128, LT, 256], BF, name="c_sb", tag="c_sb")
                s_sb = pa.tile([128, LT, 256], BF, name="s_sb", tag="s_sb")
                wch = {}

                def load_wch(c):
                    if c >= NCH or c in wch:
                        return
                    wa = paw.tile([128, DT, 512], F8, name="wchA", tag="wchA")
                    wb = paw.tile([128, DT, 2, 512], F8, name="wchB",
                                  tag="wchB")
                    wAr = wA[:, c * 512:(c + 1) * 512].rearrange(
                        "(d p) e -> p d e", p=128)
                    wBr = wB[:, c * 1024:(c + 1) * 1024].rearrange(
                        "(d p) e -> p d e", p=128)
                    wbf = wb.rearrange("p d t e -> p d (t e)")
                    for d4 in range(DT // 4):
                        sl = slice(4 * d4, 4 * d4 + 4)
                        nc.sync.dma_start(out=wa[:, sl, :], in_=wAr[:, sl, :])
                        nc.sync.dma_start(out=wbf[:, sl, :], in_=wBr[:, sl, :])
                    wch[c] = (wa, wb)

                def load_x():
                    wa = paw.tile([128, DT, 512], F8, name="wchA", tag="wchA")
                    wb = paw.tile([128, DT, 2, 512], F8, name="wchB",
                                  tag="wchB")
                    wAr = wA[:, 0:512].rearrange("(d p) e -> p d e", p=128)
                    wBr = wB[:, 0:1024].rearrange("(d p) e -> p d e", p=128)
                    wbf = wb.rearrange("p d t e -> p d (t e)")
                    xhr = xThi.rearrange("(d p) l -> p d l", p=128)
                    xlr = xTlo.rearrange("(d p) l -> p d l", p=128)
                    for d2 in range(DT // 2):
                        sl = slice(2 * d2, 2 * d2 + 2)
                        nc.sync.dma_start(out=xall[:, sl, 1, :],
                                          in_=xhr[:, sl, :])
                        nc.sync.dma_start(out=xall[:, sl, 0, :],
                                          in_=xlr[:, sl, :])
                        if d2 % 2 == 1:
                            sl4 = slice(2 * d2 - 2, 2 * d2 + 2)
                            nc.sync.dma_start(out=wa[:, sl4, :],
                                              in_=wAr[:, sl4, :])
                            nc.sync.dma_start(out=wbf[:, sl4, :],
                                              in_=wBr[:, sl4, :])
                            i2 = slice(d2 - 1, d2 + 1)
                            for t_sb, t_dr in ((c_sb, chalf), (s_sb, shalf)):
                                nc.sync.dma_start(
                                    out=t_sb[:, i2, :],
                                    in_=t_dr.rearrange("(i p) g -> p i g",
                                                       p=128)[:, i2, :])
                    # rope tables for the second half of the l range trail
                    for t_sb, t_dr in ((c_sb, chalf), (s_sb, shalf)):
                        nc.sync.dma_start(
                            out=t_sb[:, 8:LT, :],
                            in_=t_dr.rearrange("(i p) g -> p i g",
                                               p=128)[:, 8:LT, :])
                    wch[0] = (wa, wb)

                def emit_A_alpha(c, i):
                    wa, _ = wch[c]
                    ls = slice(i * 128, (i + 1) * 128)
                    pnat = P["psa"].tile([128, 512], F32, name="pnat",
                                         tag="pnat")
                    for d2 in range(DT // 2):
                        nc.tensor.matmul(
                            pnat,
                            xall[:, 2 * d2:2 * d2 + 2, 1, ls],
                            wa[:, 2 * d2:2 * d2 + 2, :],
                            start=(d2 == 0), stop=False, perf_mode=DR)
                    return pnat

                def emit_A_finish(c, i, pnat):
                    _, wb = wch[c]
                    if i == 8:
                        load_wch(c + 1)
                    ls = slice(i * 128, (i + 1) * 128)
                    for d in range(DT):
                        nc.tensor.matmul(
                            pnat,
                            xall[:, d, :, ls],
                            wb[:, d, :, :],
                            start=False, stop=(d == DT - 1), perf_mode=DR)
                    emit_A_rope(c, i, pnat)

                def emit_A_rope(c, i, pnat):
                    kind, grp = _chunk_kind(c)
                    ls = slice(i * 128, (i + 1) * 128)
                    if kind == "v":
                        nc.scalar.activation(
                            out=vsb[grp][:, i, :], in_=pnat,
                            func=mybir.ActivationFunctionType.Copy,
                            scale=1.0 / 1024.0)
                        return
                    # RoPE: per-head comps are permuted (evens | odds)
                    pv = pnat.rearrange("p (hh t z) -> p hh t z",
                                        hh=4, t=2, z=64)
                    x1 = pat.tile([128, 256], BF, name="x1", tag="x1")
                    nc.scalar.activation(
                        out=x1, in_=pv[:, :, 0, :],
                        func=mybir.ActivationFunctionType.Copy,
                        scale=1.0 / 1024.0)
                    x2 = pat.tile([128, 256], BF, name="x2", tag="x2")
                    nc.scalar.activation(
                        out=x2, in_=pv[:, :, 1, :],
                        func=mybir.ActivationFunctionType.Copy,
                        scale=1.0 / 1024.0)
                    ct = c_sb[:, i, :]
                    st = s_sb[:, i, :]
                    t1 = pat.tile([128, 256], BF, name="t1", tag="t1")
                    nc.vector.tensor_mul(t1, x1, ct)
                    t2 = pat.tile([128, 256], BF, name="t2", tag="t2")
                    nc.vector.tensor_mul(t2, x2, st)
                    t3 = pat.tile([128, 256], BF, name="t3", tag="t3")
                    nc.vector.tensor_mul(t3, x2, ct)
                    t4 = pat.tile([128, 256], BF, name="t4", tag="t4")
                    nc.vector.tensor_mul(t4, x1, st)
                    ro = pao.tile([128, 512], BF, name="ro", tag="ro")
                    rv = ro.rearrange("p (hh t z) -> p hh t z", hh=4, t=2, z=64)
                    nc.vector.tensor_sub(rv[:, :, 0, :], t1, t2)
                    nc.vector.tensor_add(rv[:, :, 1, :], t3, t4)
                    dst = qkrot[0 if kind == "q" else 1][grp]
                    nc.sync.dma_start(out=dst[ls, :], in_=ro)

                def emit_A_tile(c, i):
                    emit_A_finish(c, i, emit_A_alpha(c, i))

                def emit_A_dmajor(c, tiles):
                    # d-major across several open psum groups: every arriving
                    # x/w piece-group unlocks one alpha+2 betas per open tile
                    wa, wb = wch[c]
                    pns = [P["psa"].tile([128, 512], F32, name="pnat",
                                         tag="pnat") for _ in tiles]
                    for d2 in range(DT // 2):
                        for t, i in enumerate(tiles):
                            ls = slice(i * 128, (i + 1) * 128)
                            nc.tensor.matmul(
                                pns[t],
                                xall[:, 2 * d2:2 * d2 + 2, 1, ls],
                                wa[:, 2 * d2:2 * d2 + 2, :],
                                start=(d2 == 0), stop=False, perf_mode=DR)
                        for dd in (2 * d2, 2 * d2 + 1):
                            for t, i in enumerate(tiles):
                                ls = slice(i * 128, (i + 1) * 128)
                                nc.tensor.matmul(
                                    pns[t],
                                    xall[:, dd, :, ls],
                                    wb[:, dd, :, :],
                                    start=False, stop=(dd == DT - 1),
                                    perf_mode=DR)
                    for t, i in enumerate(tiles):
                        emit_A_rope(c, i, pns[t])

                # ---- emission: A(g0), then heads 0-3 x A(g1) ----
                if doA:
                    load_x()
                    # chunk 0 lead-in: 7 psum banks, d-major
                    with tc.tile_pool(name="psA0", bufs=8,
                                      space="PSUM") as psa0:
                        P["psa"] = psa0
                        emit_A_dmajor(0, range(8))
                        for i in range(8, LT):
                            emit_A_tile(0, i)
                with tc.tile_pool(name="psS", bufs=scb, space="PSUM") as pss, \
                     tc.tile_pool(name="psY", bufs=ypb, space="PSUM") as psy, \
                     tc.tile_pool(name="psD", bufs=1, space="PSUM") as psd:
                    P["pss"], P["psy"], P["psd"] = pss, psy, psd
                    with tc.tile_pool(name="psA", bufs=psab,
                                      space="PSUM") as psa:
                        P["psa"] = psa
                        if doA:
                            for c in (1, 2):
                                for i in range(LT):
                                    emit_A_tile(c, i)
                        g1 = [(c, i) for c in (3, 4, 5) for i in range(LT)] \
                            if doA else []
                        gi = 0
                        qts.clear()
                        for h in range(4 if doB else 0):
                            if h not in qts:
                                qts[h] = load_qk(h)
                            qt, kt = qts.pop(h)
                            for qc in range(QC):
                                emit_B_qc(h, qt, kt, qc)
                                if qc == 0 and h < 4 and doB:
                                    qts[h + 1] = load_qk(h + 1)
                                for _ in range(take):
                                    if gi < len(g1):
                                        emit_A_tile(*g1[gi])
                                        gi += 1
                        while gi < len(g1):
                            emit_A_tile(*g1[gi])
                            gi += 1

            # ---------------- heads 4-7 (qc-major) + phase C ----------
            with tc.tile_pool(name="pBy47", bufs=1) as pby47, \
                 tc.tile_pool(name="pCo", bufs=4) as pco, \
                 tc.tile_pool(name="psS3", bufs=scb, space="PSUM") as pss3, \
                 tc.tile_pool(name="psY3", bufs=ypb, space="PSUM") as psy3, \
                 tc.tile_pool(name="psD3", bufs=1, space="PSUM") as psd3, \
                 tc.tile_pool(name="psC", bufs=2, space="PSUM") as psc:
                P["pss"], P["psy"], P["psd"] = pss3, psy3, psd3
                qk47 = dict(qts)   # h4 was prefetched into the pb pool
                for h in range(5, NH if doB else 5):
                    grp, hh = h // 4, h % 4
                    qt = pby47.tile([128, L], BF, name=f"qt{h}", tag=f"qt{h}")
                    kt = pby47.tile([128, L], BF, name=f"kt{h}", tag=f"kt{h}")
                    for t, src in ((qt, qkrot[0][grp]), (kt, qkrot[1][grp])):
                        nc.sync.dma_start_transpose(
                            out=t, in_=src[:, hh * 128:(hh + 1) * 128])
                    qk47[h] = (qt, kt)
                for h in range(4, NH):
                    yts[h] = pby47.tile([128, L], BF, name=f"yt{h}",
                                        tag=f"yt{h}")
                    if not doB:
                        nc.vector.memset(yts[h], 0.0)
                wos = []
                for dd in range(NH):
                    wo = pby47.tile([128, L], BF, name=f"wo{dd}", tag=f"wo{dd}")
                    nc.sync.dma_start(
                        out=wo, in_=woT[dd * 128:(dd + 1) * 128, :])
                    wos.append(wo)

                def emit_C(e, qc):
                    op = psc.tile([128, 512], F32, name="op", tag="op")
                    for dd in range(NH):
                        nc.tensor.matmul(
                            op,
                            wos[dd][:, e * 128:(e + 1) * 128],
                            yts[dd][:, qc * 512:(qc + 1) * 512],
                            start=(dd == 0), stop=(dd == NH - 1))
                    ot = pco.tile([128, 512], BF, name="ot", tag="ot")
                    nc.vector.tensor_copy(out=ot, in_=op)
                    nc.sync.dma_start(
                        out=outT[e * 128:(e + 1) * 128,
                                 qc * 512:(qc + 1) * 512],
                        in_=ot)

                for qc in range(QC if doB else 0):
                    for h in range(4, NH):
                        emit_B_qc(h, qk47[h][0], qk47[h][1], qc)
                    if doC and qc > 0:
                        for e in range(DT):
                            emit_C(e, qc - 1)
                if doC:
                    for qc in ([3] if doB else range(QC)):
                        for e in range(DT):
                            emit_C(e, qc)
    nc.compile()
    return nc


_NC_CACHE = None


def _get_program():
    global _NC_CACHE
    if _NC_CACHE is None:
        _NC_CACHE = build_program()
    return _NC_CACHE


def _f8(a):
    return np.clip(np.asarray(a, np.float64), -240.0, 240.0).astype(F8NP)


# within each head's 128 comps: evens first, then odds
_PERM512 = np.concatenate(
    [np.concatenate([np.arange(h * 128, (h + 1) * 128, 2),
                     np.arange(h * 128 + 1, (h + 1) * 128, 2)])
     for h in range(4)])


def _host_inputs(x, w_qkv, w_o):
    inv = 1.0 / (ROPE_BASE ** (np.arange(0, HD, 2, dtype=np.float64) / HD))
    ang = np.arange(L, dtype=np.float64)[:, None] * inv[None, :]
    chalf = np.tile(np.cos(ang), (1, 4)).astype(BF16)          # [L, 256]
    shalf = np.tile(np.sin(ang), (1, 4)).astype(BF16)
    p = np.arange(128)[:, None]
    f = np.arange(128)[None, :]
    tri = (p <= f).astype(np.float16)                          # [128, 128]

    xhi_all, xlo_all = [], []
    for b in range(B):
        xb = np.asarray(x[b], np.float64)                      # [L, D]
        xh = _f8(xb)
        xl = _f8(16.0 * (xb - xh.astype(np.float64)))
        xhi_all.append(np.ascontiguousarray(xh.T))
        xlo_all.append(np.ascontiguousarray(xl.T))

    in_maps = []
    for c in range(8):
        b, g = c % 4, c // 4
        qr = w_qkv[g * DL:(g + 1) * DL]
        kr = w_qkv[D + g * DL:D + (g + 1) * DL]
        vr = w_qkv[2 * D + g * DL:2 * D + (g + 1) * DL]
        qp = qr[np.concatenate([_PERM512, 512 + _PERM512])]
        kp = kr[np.concatenate([_PERM512, 512 + _PERM512])]
        wloc = np.concatenate([qp[:512], kp[:512], vr[:512],
                               qp[512:], kp[512:], vr[512:]],
                              axis=0).astype(np.float64)       # [3DL, D]
        w64h = _f8(64.0 * wloc)
        w1024h = (w64h.astype(np.float64) * 16.0).astype(F8NP)  # exact shift
        w16l = _f8(16.0 * (64.0 * wloc - w64h.astype(np.float64)))
        wAh = np.ascontiguousarray(w1024h.T)                    # [D, 3DL]
        w64hT, w16lT = w64h.T, w16l.T                           # [D, 3DL]
        wBh = np.ascontiguousarray(np.concatenate(
            [np.stack([w64hT[:, c2 * 512:(c2 + 1) * 512],
                       w16lT[:, c2 * 512:(c2 + 1) * 512]],
                      axis=1).reshape(D, 1024)
             for c2 in range(NCH)], axis=1))                    # [D, 6DL]
        in_maps.append({
            "xThi": xhi_all[b],
            "xTlo": xlo_all[b],
            "wA": wAh,
            "wB": wBh,
            "woT": np.ascontiguousarray(
                w_o[:, g * DL:(g + 1) * DL].T).astype(BF16),
            "chalf": chalf,
            "shalf": shalf,
            "tri": tri,
        })
    return in_maps


def kernel(x, w_qkv, w_o, _trace=False):
    x = np.asarray(x, dtype=np.float32)
    w_qkv = np.asarray(w_qkv, dtype=np.float32)
    w_o = np.asarray(w_o, dtype=np.float32)
    nc = _get_program()
    in_maps = _host_inputs(x, w_qkv, w_o)
    res = run_bass_kernel_spmd(nc, in_maps, core_ids=list(range(8)),
                               trace=_trace)
    kernel.last_result = res
    parts = [r["outT"].astype(np.float32) for r in res.results]
    out = np.empty((B, L, D), dtype=np.float32)
    for b in range(B):
        out[b] = (parts[b] + parts[b + 4]).T
    return out


# revision 4
# speedup vs baseline: 1.0973x; 1.0163x over previous
"""MHA (RoPE + causal softmax attention + out-proj) on 8 NeuronCores — v3.

Sharding: DP4 x TP2 (core c: batch c % 4, head-group c // 4; 8 heads/core).
Host sums the two head-group partial outputs per batch and transposes.

Key structure (tuned against the TimelineSim cost model):
  * Phase A (QKV) matmuls run in fp8 e4m3 DoubleRow with a hi/lo split of
    both operands (x ~ xh + xl/16, 64w ~ wh + wl/16, lo*lo dropped):
    3 DR instructions per d-tile pair = 0.75x the bf16 instruction cost at
    better-than-bf16 accuracy. PSUM carries 1024*qkv; the 2^-10 unscale is
    folded into the RoPE-cast / v copy scales.
  * q/k comps are host-permuted to (evens | odds) within each head so RoPE
    reads contiguous PSUM blocks: 2 ACT casts + 6 DVE bf16 ops (4x mode).
  * Softmax: exp(alpha*s - 8ln2) -> fp16 exp tiles; denominator accumulated
    with DVE adds + ONE ones-matmul per (head, qc) instead of a ones-matmul
    per k-tile. Causal diagonal tiles are column-trimmed; a single [128,128]
    triangle mask remains.
  * v never round-trips DRAM (PSUM -> SBUF fp16 copy, resident).
  * Emission interleaves head-group-1 QKV tiles into the attention loop of
    head-group-0 so the exp-bound stretch of attention overlaps the
    PE-bound QKV GEMM instead of stalling the tensor engine.
"""

import numpy as np
import ml_dtypes

import concourse.bass as bass
import concourse.tile as tile
import concourse.mybir as mybir
from concourse import bacc
from concourse.bass_utils import run_bass_kernel_spmd

BF16 = ml_dtypes.bfloat16
F8NP = ml_dtypes.float8_e4m3
F32 = mybir.dt.float32
BF = mybir.dt.bfloat16
F16 = mybir.dt.float16
F8 = mybir.dt.float8e4
DR = mybir.MatmulPerfMode.DoubleRow

B, L, D, H, HD = 4, 2048, 2048, 16, 128
NH = 8                      # heads per core
DL = NH * HD                # 1024 local head dims
ROPE_BASE = 10000.0
ALPHA = float(HD) ** -0.5
EXP_BIAS = -8.0 * float(np.log(2.0))   # exp(a*s - 8ln2): keeps fp16 sums safe

LT = L // 128               # 16 L-tiles
DT = D // 128               # 16 D(contract)-tiles
NCH = 6                     # qkv chunks of 512 comps: q03,k03,v03,q47,k47,v47
QC = L // 512               # 4 q-chunks of 512
KT = L // 128               # 16 k-tiles


def _chunk_kind(c):
    # chunk order: q(heads0-3), k(0-3), v(0-3), q(4-7), k(4-7), v(4-7)
    return ("q", "k", "v")[c % 3], c // 3


def build_program(phases="ABC", la=3, scb=3, ypb=1, psab=3, patb=1, paob=2,
                  pbab=5, take=3):
    nc = bacc.Bacc("TRN2", target_bir_lowering=False, debug=False, num_devices=8)

    # x hi/lo fp8 planes: hi = fp8(x), lo = fp8(16*(x-hi))
    xThi = nc.dram_tensor("xThi", [D, L], F8, kind="ExternalInput").ap()
    xTlo = nc.dram_tensor("xTlo", [D, L], F8, kind="ExternalInput").ap()
    # w planes: wA = fp8(16*w64_hi) [D, 3DL]; wB [D, 2*3DL]: per 512-chunk,
    # 1024 cols = (w64_hi 512 | w16_lo 512)
    wA = nc.dram_tensor("wA", [D, 3 * DL], F8, kind="ExternalInput").ap()
    wB = nc.dram_tensor("wB", [D, 6 * DL], F8, kind="ExternalInput").ap()
    woA = nc.dram_tensor("woA", [DL, L], F8, kind="ExternalInput").ap()
    woB = nc.dram_tensor("woB", [DL, 2 * L], F8, kind="ExternalInput").ap()
    chalf = nc.dram_tensor("chalf", [L, 256], BF, kind="ExternalInput").ap()
    shalf = nc.dram_tensor("shalf", [L, 256], BF, kind="ExternalInput").ap()
    tri = nc.dram_tensor("tri", [128, 128], F16, kind="ExternalInput").ap()
    outT = nc.dram_tensor("outT", [D, L], BF, kind="ExternalOutput").ap()

    # DRAM staging for rotated q/k, split per head-group so group-0 attention
    # does not depend on group-1 writes
    qkrot = [[nc.dram_tensor(f"{nm}rot{g}", [L, 512], BF, kind="Internal").ap()
              for g in range(2)] for nm in ("q", "k")]

    doA = "A" in phases
    doB = "B" in phases
    doC = "C" in phases

    with tile.TileContext(nc) as tc:
        outer_cm = tc.tile_pool(name="outer", bufs=1)
        pb_cm = tc.tile_pool(name="pBqk", bufs=2, side="right")
        pbm_cm = tc.tile_pool(name="pBm", bufs=1, side="right")
        pby03_cm = tc.tile_pool(name="pBy03", bufs=1)
        pba_cm = tc.tile_pool(name="pBa", bufs=pbab)
        pbr_cm = tc.tile_pool(name="pBr", bufs=1)
        pbv_cm = tc.tile_pool(name="pBv", bufs=1)
        pbd_cm = tc.tile_pool(name="pBd", bufs=2)
        P = {}
        with outer_cm as outer, pb_cm as pb, pbm_cm as pbm, \
             pby03_cm as pby03, pba_cm as pba, pbr_cm as pbr, \
             pbv_cm as pbv, pbd_cm as pbd:
            # v for both head groups, resident SBUF: [128(kpos), LT, 512]
            vsb = [outer.tile([128, LT, 512], F16, name=f"vsb{g}",
                              tag=f"vsb{g}") for g in range(2)]
            ones128 = outer.tile([128, 128], F16, name="ones128", tag="oc")
            nc.vector.memset(ones128, 1.0)
            ebias = outer.tile([128, 1], F32, name="ebias", tag="ebias")
            nc.vector.memset(ebias, EXP_BIAS)
            trit = pbm.tile([128, 128], F16, name="tri", tag="tri")
            nc.sync.dma_start(out=trit, in_=tri)

            qts = {}
            yall = {}
            yall[0] = pby03.tile([128, 4, 2, L], F8, name="yall0", tag="yall0")
            if not doB:
                nc.vector.memset(yall[0], 0.0)

            # ---------------- phase B helpers ----------------
            def load_qk(h):
                grp, hh = h // 4, h % 4
                qt = pb.tile([128, L], BF, name="qt", tag="qt")
                kt = pb.tile([128, L], BF, name="kt", tag="kt")
                for t, src in ((qt, qkrot[0][grp]), (kt, qkrot[1][grp])):
                    nc.sync.dma_start_transpose(
                        out=t, in_=src[:, hh * 128:(hh + 1) * 128])
                return qt, kt

            def emit_B_qc(h, qt, kt, qc):
                grp, hh = h // 4, h % 4
                nkt = 4 * qc + 4
                ypsum = P["psy"].tile([128, 512], F32, name="ypsum", tag="yp")
                dacc = pbd.tile([128, 512], F16, name="dacc", tag="dacc")
                ats = {}

                def emit_score(j):
                    m = j - 4 * qc
                    off = 128 * m if m > 0 else 0
                    w = 512 - off
                    sc = P["pss"].tile([128, 512], F32, name="sc", tag="sc")
                    nc.tensor.matmul(
                        sc[:, 0:w], kt[:, j * 128:(j + 1) * 128],
                        qt[:, qc * 512 + off:(qc + 1) * 512],
                        start=True, stop=True)
                    at = pba.tile([128, 512], F16, name="at", tag="at")
                    nc.scalar.activation(
                        out=at[:, 0:w], in_=sc[:, 0:w],
                        func=mybir.ActivationFunctionType.Exp,
                        scale=ALPHA, bias=ebias)
                    if m >= 0:
                        nc.vector.tensor_mul(at[:, 0:128], at[:, 0:128], trit)
                    ats[j] = (at, off, w)

                for j in range(min(la, nkt)):
                    emit_score(j)
                prev = None
                for j in range(nkt):
                    if j + la < nkt:
                        emit_score(j + la)
                    at, off, w = ats.pop(j)
                    nc.tensor.matmul(
                        ypsum[:, off:512],
                        vsb[grp][:, j, hh * 128:(hh + 1) * 128],
                        at[:, 0:w],
                        start=(j == 0), stop=(j == nkt - 1),
                        skip_group_check=True)
                    if j == 0:
                        prev = at
                    elif j == 1:
                        if qc == 0:
                            nc.vector.tensor_copy(
                                out=dacc[:, 0:128], in_=prev[:, 0:128])
                            nc.vector.tensor_add(
                                dacc[:, 128:512], prev[:, 128:512],
                                at[:, 0:w])
                        else:
                            nc.vector.tensor_add(dacc, prev, at)
                    else:
                        nc.vector.tensor_add(
                            dacc[:, off:512], dacc[:, off:512], at[:, 0:w])
                dpsum = P["psd"].tile([128, 512], F32, name="dpsum", tag="dp")
                nc.tensor.matmul(dpsum, ones128, dacc, start=True, stop=True)
                rbs = pbr.tile([128, 512], BF, name="rbs", tag="rbs")
                with nc.allow_low_precision("softmax recip bf16"):
                    nc.vector.reciprocal(out=rbs, in_=dpsum)
                qsl = slice(qc * 512, (qc + 1) * 512)
                ya = yall[grp]
                yb = pbv.tile([128, 512], BF, name="yb", tag="yb")
                nc.vector.tensor_mul(yb, ypsum, rbs)
                nc.vector.tensor_copy(out=ya[:, hh, 1, qsl], in_=yb)
                yd = pbv.tile([128, 512], BF, name="yd", tag="yd")
                nc.vector.tensor_sub(yd, yb, ya[:, hh, 1, qsl])
                nc.vector.tensor_scalar_mul(ya[:, hh, 0, qsl], yd, 16.0)

            # ---------------- phase A scope + interleave ----------------
            with tc.tile_pool(name="pA", bufs=1) as pa, \
                 tc.tile_pool(name="pAw", bufs=2) as paw, \
                 tc.tile_pool(name="pAt", bufs=patb) as pat, \
                 tc.tile_pool(name="pAo", bufs=paob) as pao:
                xall = pa.tile([128, DT, 2, L], F8, name="xall", tag="xall")
                c_sb = pa.tile([128, LT, 256], BF, name="c_sb", tag="c_sb")
                s_sb = pa.tile([128, LT, 256], BF, name="s_sb", tag="s_sb")
                wch = {}

                def load_wch(c):
                    if c >= NCH or c in wch:
                        return
                    wa = paw.tile([128, DT, 512], F8, name="wchA", tag="wchA")
                    wb = paw.tile([128, DT, 2, 512], F8, name="wchB",
                                  tag="wchB")
                    wAr = wA[:, c * 512:(c + 1) * 512].rearrange(
                        "(d p) e -> p d e", p=128)
                    wBr = wB[:, c * 1024:(c + 1) * 1024].rearrange(
                        "(d p) e -> p d e", p=128)
                    wbf = wb.rearrange("p d t e -> p d (t e)")
                    for d4 in range(DT // 4):
                        sl = slice(4 * d4, 4 * d4 + 4)
                        nc.sync.dma_start(out=wa[:, sl, :], in_=wAr[:, sl, :])
                        nc.sync.dma_start(out=wbf[:, sl, :], in_=wBr[:, sl, :])
                    wch[c] = (wa, wb)

                def load_x():
                    wa = paw.tile([128, DT, 512], F8, name="wchA", tag="wchA")
                    wb = paw.tile([128, DT, 2, 512], F8, name="wchB",
                                  tag="wchB")
                    wAr = wA[:, 0:512].rearrange("(d p) e -> p d e", p=128)
                    wBr = wB[:, 0:1024].rearrange("(d p) e -> p d e", p=128)
                    wbf = wb.rearrange("p d t e -> p d (t e)")
                    xhr = xThi.rearrange("(d p) l -> p d l", p=128)
                    xlr = xTlo.rearrange("(d p) l -> p d l", p=128)
                    for d2 in range(DT // 2):
                        sl = slice(2 * d2, 2 * d2 + 2)
                        nc.sync.dma_start(out=xall[:, sl, 1, :],
                                          in_=xhr[:, sl, :])
                        nc.sync.dma_start(out=xall[:, sl, 0, :],
                                          in_=xlr[:, sl, :])
                        if d2 % 2 == 1:
                            sl4 = slice(2 * d2 - 2, 2 * d2 + 2)
                            nc.sync.dma_start(out=wa[:, sl4, :],
                                              in_=wAr[:, sl4, :])
                            nc.sync.dma_start(out=wbf[:, sl4, :],
                                              in_=wBr[:, sl4, :])
                            i2 = slice(d2 - 1, d2 + 1)
                            for t_sb, t_dr in ((c_sb, chalf), (s_sb, shalf)):
                                nc.sync.dma_start(
                                    out=t_sb[:, i2, :],
                                    in_=t_dr.rearrange("(i p) g -> p i g",
                                                       p=128)[:, i2, :])
                    # rope tables for the second half of the l range trail
                    for t_sb, t_dr in ((c_sb, chalf), (s_sb, shalf)):
                        nc.sync.dma_start(
                            out=t_sb[:, 8:LT, :],
                            in_=t_dr.rearrange("(i p) g -> p i g",
                                               p=128)[:, 8:LT, :])
                    wch[0] = (wa, wb)

                def emit_A_alpha(c, i):
                    wa, _ = wch[c]
                    ls = slice(i * 128, (i + 1) * 128)
                    pnat = P["psa"].tile([128, 512], F32, name="pnat",
                                         tag="pnat")
                    for d2 in range(DT // 2):
                        nc.tensor.matmul(
                            pnat,
                            xall[:, 2 * d2:2 * d2 + 2, 1, ls],
                            wa[:, 2 * d2:2 * d2 + 2, :],
                            start=(d2 == 0), stop=False, perf_mode=DR)
                    return pnat

                def emit_A_finish(c, i, pnat):
                    _, wb = wch[c]
                    if i == 8:
                        load_wch(c + 1)
                    ls = slice(i * 128, (i + 1) * 128)
                    for d in range(DT):
                        nc.tensor.matmul(
                            pnat,
                            xall[:, d, :, ls],
                            wb[:, d, :, :],
                            start=False, stop=(d == DT - 1), perf_mode=DR)
                    emit_A_rope(c, i, pnat)

                def emit_A_rope(c, i, pnat):
                    kind, grp = _chunk_kind(c)
                    ls = slice(i * 128, (i + 1) * 128)
                    if kind == "v":
                        nc.scalar.activation(
                            out=vsb[grp][:, i, :], in_=pnat,
                            func=mybir.ActivationFunctionType.Copy,
                            scale=1.0 / 1024.0)
                        return
                    # RoPE: per-head comps are permuted (evens | odds)
                    pv = pnat.rearrange("p (hh t z) -> p hh t z",
                                        hh=4, t=2, z=64)
                    x1 = pat.tile([128, 256], BF, name="x1", tag="x1")
                    nc.scalar.activation(
                        out=x1, in_=pv[:, :, 0, :],
                        func=mybir.ActivationFunctionType.Copy,
                        scale=1.0 / 1024.0)
                    x2 = pat.tile([128, 256], BF, name="x2", tag="x2")
                    nc.scalar.activation(
                        out=x2, in_=pv[:, :, 1, :],
                        func=mybir.ActivationFunctionType.Copy,
                        scale=1.0 / 1024.0)
                    ct = c_sb[:, i, :]
                    st = s_sb[:, i, :]
                    t1 = pat.tile([128, 256], BF, name="t1", tag="t1")
                    nc.vector.tensor_mul(t1, x1, ct)
                    t2 = pat.tile([128, 256], BF, name="t2", tag="t2")
                    nc.vector.tensor_mul(t2, x2, st)
                    t3 = pat.tile([128, 256], BF, name="t3", tag="t3")
                    nc.vector.tensor_mul(t3, x2, ct)
                    t4 = pat.tile([128, 256], BF, name="t4", tag="t4")
                    nc.vector.tensor_mul(t4, x1, st)
                    ro = pao.tile([128, 512], BF, name="ro", tag="ro")
                    rv = ro.rearrange("p (hh t z) -> p hh t z", hh=4, t=2, z=64)
                    nc.vector.tensor_sub(rv[:, :, 0, :], t1, t2)
                    nc.vector.tensor_add(rv[:, :, 1, :], t3, t4)
                    dst = qkrot[0 if kind == "q" else 1][grp]
                    nc.sync.dma_start(out=dst[ls, :], in_=ro)

                def emit_A_tile(c, i):
                    emit_A_finish(c, i, emit_A_alpha(c, i))

                def emit_A_dmajor(c, tiles):
                    # d-major across several open psum groups: every arriving
                    # x/w piece-group unlocks one alpha+2 betas per open tile
                    wa, wb = wch[c]
                    pns = [P["psa"].tile([128, 512], F32, name="pnat",
                                         tag="pnat") for _ in tiles]
                    for d2 in range(DT // 2):
                        for t, i in enumerate(tiles):
                            ls = slice(i * 128, (i + 1) * 128)
                            nc.tensor.matmul(
                                pns[t],
                                xall[:, 2 * d2:2 * d2 + 2, 1, ls],
                                wa[:, 2 * d2:2 * d2 + 2, :],
                                start=(d2 == 0), stop=False, perf_mode=DR)
                        for dd in (2 * d2, 2 * d2 + 1):
                            for t, i in enumerate(tiles):
                                ls = slice(i * 128, (i + 1) * 128)
                                nc.tensor.matmul(
                                    pns[t],
                                    xall[:, dd, :, ls],
                                    wb[:, dd, :, :],
                                    start=False, stop=(dd == DT - 1),
                                    perf_mode=DR)
                    for t, i in enumerate(tiles):
                        emit_A_rope(c, i, pns[t])

                # ---- emission: A(g0), then heads 0-3 x A(g1) ----
                if doA:
                    load_x()
                    # chunk 0 lead-in: 7 psum banks, d-major
                    with tc.tile_pool(name="psA0", bufs=8,
                                      space="PSUM") as psa0:
                        P["psa"] = psa0
                        emit_A_dmajor(0, range(8))
                        for i in range(8, LT):
                            emit_A_tile(0, i)
                with tc.tile_pool(name="psS", bufs=scb, space="PSUM") as pss, \
                     tc.tile_pool(name="psY", bufs=ypb, space="PSUM") as psy, \
                     tc.tile_pool(name="psD", bufs=1, space="PSUM") as psd:
                    P["pss"], P["psy"], P["psd"] = pss, psy, psd
                    with tc.tile_pool(name="psA", bufs=psab,
                                      space="PSUM") as psa:
                        P["psa"] = psa
                        if doA:
                            for c in (1, 2):
                                for i in range(LT):
                                    emit_A_tile(c, i)
                        g1 = [(c, i) for c in (3, 4, 5) for i in range(LT)] \
                            if doA else []
                        gi = 0
                        qts.clear()
                        for h in range(4 if doB else 0):
                            if h not in qts:
                                qts[h] = load_qk(h)
                            qt, kt = qts.pop(h)
                            for qc in range(QC):
                                emit_B_qc(h, qt, kt, qc)
                                if qc == 0 and h < 4 and doB:
                                    qts[h + 1] = load_qk(h + 1)
                                for _ in range(take):
                                    if gi < len(g1):
                                        emit_A_tile(*g1[gi])
                                        gi += 1
                        while gi < len(g1):
                            emit_A_tile(*g1[gi])
                            gi += 1

            # ---------------- heads 4-7 (qc-major) + phase C ----------
            with tc.tile_pool(name="pBy47", bufs=1) as pby47, \
                 tc.tile_pool(name="pCo", bufs=4) as pco, \
                 tc.tile_pool(name="psS3", bufs=scb, space="PSUM") as pss3, \
                 tc.tile_pool(name="psY3", bufs=ypb, space="PSUM") as psy3, \
                 tc.tile_pool(name="psD3", bufs=1, space="PSUM") as psd3, \
                 tc.tile_pool(name="psC", bufs=2, space="PSUM") as psc:
                P["pss"], P["psy"], P["psd"] = pss3, psy3, psd3
                qk47 = dict(qts)   # h4 was prefetched into the pb pool
                for h in range(5, NH if doB else 5):
                    grp, hh = h // 4, h % 4
                    qt = pby47.tile([128, L], BF, name=f"qt{h}", tag=f"qt{h}")
                    kt = pby47.tile([128, L], BF, name=f"kt{h}", tag=f"kt{h}")
                    for t, src in ((qt, qkrot[0][grp]), (kt, qkrot[1][grp])):
                        nc.sync.dma_start_transpose(
                            out=t, in_=src[:, hh * 128:(hh + 1) * 128])
                    qk47[h] = (qt, kt)
                yall[1] = pby47.tile([128, 4, 2, L], F8, name="yall1",
                                     tag="yall1")
                if not doB:
                    nc.vector.memset(yall[1], 0.0)
                woat = pby47.tile([128, NH, L], F8, name="woat", tag="woat")
                nc.sync.dma_start(
                    out=woat, in_=woA.rearrange("(dd p) e -> p dd e", p=128))
                wobt = pby47.tile([128, NH, 2, L], F8, name="wobt", tag="wobt")
                nc.sync.dma_start(
                    out=wobt.rearrange("p dd t e -> p dd (t e)"),
                    in_=woB.rearrange("(dd p) e -> p dd e", p=128))

                def emit_C(e, qc):
                    esl = slice(e * 128, (e + 1) * 128)
                    qsl = slice(qc * 512, (qc + 1) * 512)
                    op = psc.tile([128, 512], F32, name="op", tag="op")
                    for g in range(2):
                        for p2 in range(2):
                            dd = 4 * g + 2 * p2
                            nc.tensor.matmul(
                                op, woat[:, dd:dd + 2, esl],
                                yall[g][:, 2 * p2:2 * p2 + 2, 1, qsl],
                                start=(g == 0 and p2 == 0), stop=False,
                                perf_mode=DR)
                    for g in range(2):
                        for hh in range(4):
                            nc.tensor.matmul(
                                op, wobt[:, 4 * g + hh, :, esl],
                                yall[g][:, hh, :, qsl],
                                start=False, stop=(g == 1 and hh == 3),
                                perf_mode=DR)
                    ot = pco.tile([128, 512], BF, name="ot", tag="ot")
                    nc.scalar.activation(
                        out=ot, in_=op,
                        func=mybir.ActivationFunctionType.Copy,
                        scale=1.0 / 1024.0)
                    nc.sync.dma_start(
                        out=outT[e * 128:(e + 1) * 128,
                                 qc * 512:(qc + 1) * 512],
                        in_=ot)

                for qc in range(QC if doB else 0):
                    for h in range(4, NH):
                        emit_B_qc(h, qk47[h][0], qk47[h][1], qc)
                    if doC and qc > 0:
                        for e in range(DT):
                            emit_C(e, qc - 1)
                if doC:
                    for qc in ([3] if doB else range(QC)):
                        for e in range(DT):
                            emit_C(e, qc)
    nc.compile()
    return nc


_NC_CACHE = None


def _get_program():
    global _NC_CACHE
    if _NC_CACHE is None:
        _NC_CACHE = build_program()
    return _NC_CACHE


def _f8(a):
    return np.clip(np.asarray(a, np.float64), -240.0, 240.0).astype(F8NP)


# within each head's 128 comps: evens first, then odds
_PERM512 = np.concatenate(
    [np.concatenate([np.arange(h * 128, (h + 1) * 128, 2),
                     np.arange(h * 128 + 1, (h + 1) * 128, 2)])
     for h in range(4)])


def _host_inputs(x, w_qkv, w_o):
    inv = 1.0 / (ROPE_BASE ** (np.arange(0, HD, 2, dtype=np.float64) / HD))
    ang = np.arange(L, dtype=np.float64)[:, None] * inv[None, :]
    chalf = np.tile(np.cos(ang), (1, 4)).astype(BF16)          # [L, 256]
    shalf = np.tile(np.sin(ang), (1, 4)).astype(BF16)
    p = np.arange(128)[:, None]
    f = np.arange(128)[None, :]
    tri = (p <= f).astype(np.float16)                          # [128, 128]

    xhi_all, xlo_all = [], []
    for b in range(B):
        xb = np.asarray(x[b], np.float64)                      # [L, D]
        xh = _f8(xb)
        xl = _f8(16.0 * (xb - xh.astype(np.float64)))
        xhi_all.append(np.ascontiguousarray(xh.T))
        xlo_all.append(np.ascontiguousarray(xl.T))

    in_maps = []
    for c in range(8):
        b, g = c % 4, c // 4
        qr = w_qkv[g * DL:(g + 1) * DL]
        kr = w_qkv[D + g * DL:D + (g + 1) * DL]
        vr = w_qkv[2 * D + g * DL:2 * D + (g + 1) * DL]
        qp = qr[np.concatenate([_PERM512, 512 + _PERM512])]
        kp = kr[np.concatenate([_PERM512, 512 + _PERM512])]
        wloc = np.concatenate([qp[:512], kp[:512], vr[:512],
                               qp[512:], kp[512:], vr[512:]],
                              axis=0).astype(np.float64)       # [3DL, D]
        w64h = _f8(64.0 * wloc)
        w1024h = (w64h.astype(np.float64) * 16.0).astype(F8NP)  # exact shift
        w16l = _f8(16.0 * (64.0 * wloc - w64h.astype(np.float64)))
        wAh = np.ascontiguousarray(w1024h.T)                    # [D, 3DL]
        w64hT, w16lT = w64h.T, w16l.T                           # [D, 3DL]
        wBh = np.ascontiguousarray(np.concatenate(
            [np.stack([w64hT[:, c2 * 512:(c2 + 1) * 512],
                       w16lT[:, c2 * 512:(c2 + 1) * 512]],
                      axis=1).reshape(D, 1024)
             for c2 in range(NCH)], axis=1))                    # [D, 6DL]
        woT64 = 64.0 * w_o[:, g * DL:(g + 1) * DL].T.astype(np.float64)
        wo64h = _f8(woT64)
        woAh = np.ascontiguousarray(
            (wo64h.astype(np.float64) * 16.0).astype(F8NP))      # [DL, L]
        wo16l = _f8(16.0 * (woT64 - wo64h.astype(np.float64)))
        woBh = np.ascontiguousarray(
            np.concatenate([wo64h[:, None, :], wo16l[:, None, :]],
                           axis=1).reshape(DL, 2 * L))
        in_maps.append({
            "xThi": xhi_all[b],
            "xTlo": xlo_all[b],
            "wA": wAh,
            "wB": wBh,
            "woA": woAh,
            "woB": woBh,
            "chalf": chalf,
            "shalf": shalf,
            "tri": tri,
        })
    return in_maps


def kernel(x, w_qkv, w_o, _trace=False):
    x = np.asarray(x, dtype=np.float32)
    w_qkv = np.asarray(w_qkv, dtype=np.float32)
    w_o = np.asarray(w_o, dtype=np.float32)
    nc = _get_program()
    in_maps = _host_inputs(x, w_qkv, w_o)
    res = run_bass_kernel_spmd(nc, in_maps, core_ids=list(range(8)),
                               trace=_trace)
    kernel.last_result = res
    parts = [r["outT"].astype(np.float32) for r in res.results]
    out = np.empty((B, L, D), dtype=np.float32)
    for b in range(B):
        out[b] = (parts[b] + parts[b + 4]).T
    return out


# revision 5
# speedup vs baseline: 1.0982x; 1.0008x over previous
"""MHA (RoPE + causal softmax attention + out-proj) on 8 NeuronCores — v3.

Sharding: DP4 x TP2 (core c: batch c % 4, head-group c // 4; 8 heads/core).
Host sums the two head-group partial outputs per batch and transposes.

Key structure (tuned against the TimelineSim cost model):
  * Phase A (QKV) matmuls run in fp8 e4m3 DoubleRow with a hi/lo split of
    both operands (x ~ xh + xl/16, 64w ~ wh + wl/16, lo*lo dropped):
    3 DR instructions per d-tile pair = 0.75x the bf16 instruction cost at
    better-than-bf16 accuracy. PSUM carries 1024*qkv; the 2^-10 unscale is
    folded into the RoPE-cast / v copy scales.
  * q/k comps are host-permuted to (evens | odds) within each head so RoPE
    reads contiguous PSUM blocks: 2 ACT casts + 6 DVE bf16 ops (4x mode).
  * Softmax: exp(alpha*s - 8ln2) -> fp16 exp tiles; denominator accumulated
    with DVE adds + ONE ones-matmul per (head, qc) instead of a ones-matmul
    per k-tile. Causal diagonal tiles are column-trimmed; a single [128,128]
    triangle mask remains.
  * v never round-trips DRAM (PSUM -> SBUF fp16 copy, resident).
  * Emission interleaves head-group-1 QKV tiles into the attention loop of
    head-group-0 so the exp-bound stretch of attention overlaps the
    PE-bound QKV GEMM instead of stalling the tensor engine.
"""

import numpy as np
import ml_dtypes

import concourse.bass as bass
import concourse.tile as tile
import concourse.mybir as mybir
from concourse import bacc
from concourse.bass_utils import run_bass_kernel_spmd

BF16 = ml_dtypes.bfloat16
F8NP = ml_dtypes.float8_e4m3
F32 = mybir.dt.float32
BF = mybir.dt.bfloat16
F16 = mybir.dt.float16
F8 = mybir.dt.float8e4
DR = mybir.MatmulPerfMode.DoubleRow

B, L, D, H, HD = 4, 2048, 2048, 16, 128
NH = 8                      # heads per core
DL = NH * HD                # 1024 local head dims
ROPE_BASE = 10000.0
ALPHA = float(HD) ** -0.5
EXP_BIAS = -8.0 * float(np.log(2.0))   # exp(a*s - 8ln2): keeps fp16 sums safe

LT = L // 128               # 16 L-tiles
DT = D // 128               # 16 D(contract)-tiles
NCH = 6                     # qkv chunks of 512 comps: q03,k03,v03,q47,k47,v47
QC = L // 512               # 4 q-chunks of 512
KT = L // 128               # 16 k-tiles


def _chunk_kind(c):
    # chunk order: q(heads0-3), k(0-3), v(0-3), q(4-7), k(4-7), v(4-7)
    return ("q", "k", "v")[c % 3], c // 3


def build_program(phases="ABC", la=3, scb=3, ypb=1, psab=3, patb=1, paob=2,
                  pbab=5, take=4):
    nc = bacc.Bacc("TRN2", target_bir_lowering=False, debug=False, num_devices=8)

    # x hi/lo fp8 planes: hi = fp8(x), lo = fp8(16*(x-hi))
    xThi = nc.dram_tensor("xThi", [D, L], F8, kind="ExternalInput").ap()
    xTlo = nc.dram_tensor("xTlo", [D, L], F8, kind="ExternalInput").ap()
    # w planes: wA = fp8(16*w64_hi) [D, 3DL]; wB [D, 2*3DL]: per 512-chunk,
    # 1024 cols = (w64_hi 512 | w16_lo 512)
    wA = nc.dram_tensor("wA", [D, 3 * DL], F8, kind="ExternalInput").ap()
    wB = nc.dram_tensor("wB", [D, 6 * DL], F8, kind="ExternalInput").ap()
    woA = nc.dram_tensor("woA", [DL, L], F8, kind="ExternalInput").ap()
    woB = nc.dram_tensor("woB", [DL, 2 * L], F8, kind="ExternalInput").ap()
    chalf = nc.dram_tensor("chalf", [L, 256], BF, kind="ExternalInput").ap()
    shalf = nc.dram_tensor("shalf", [L, 256], BF, kind="ExternalInput").ap()
    tri = nc.dram_tensor("tri", [128, 128], F16, kind="ExternalInput").ap()
    outT = nc.dram_tensor("outT", [D, L], BF, kind="ExternalOutput").ap()

    # DRAM staging for rotated q/k, split per head-group so group-0 attention
    # does not depend on group-1 writes
    qkrot = [[nc.dram_tensor(f"{nm}rot{g}", [L, 512], BF, kind="Internal").ap()
              for g in range(2)] for nm in ("q", "k")]

    doA = "A" in phases
    doB = "B" in phases
    doC = "C" in phases

    with tile.TileContext(nc) as tc:
        outer_cm = tc.tile_pool(name="outer", bufs=1)
        pb_cm = tc.tile_pool(name="pBqk", bufs=2, side="right")
        pbm_cm = tc.tile_pool(name="pBm", bufs=1, side="right")
        pby03_cm = tc.tile_pool(name="pBy03", bufs=1)
        pba_cm = tc.tile_pool(name="pBa", bufs=pbab)
        pbr_cm = tc.tile_pool(name="pBr", bufs=1)
        pbv_cm = tc.tile_pool(name="pBv", bufs=1)
        pbd_cm = tc.tile_pool(name="pBd", bufs=2)
        P = {}
        with outer_cm as outer, pb_cm as pb, pbm_cm as pbm, \
             pby03_cm as pby03, pba_cm as pba, pbr_cm as pbr, \
             pbv_cm as pbv, pbd_cm as pbd:
            # v for both head groups, resident SBUF: [128(kpos), LT, 512]
            vsb = [outer.tile([128, LT, 512], F16, name=f"vsb{g}",
                              tag=f"vsb{g}") for g in range(2)]
            ones128 = outer.tile([128, 128], F16, name="ones128", tag="oc")
            nc.vector.memset(ones128, 1.0)
            ebias = outer.tile([128, 1], F32, name="ebias", tag="ebias")
            nc.vector.memset(ebias, EXP_BIAS)
            trit = pbm.tile([128, 128], F16, name="tri", tag="tri")
            nc.sync.dma_start(out=trit, in_=tri)

            qts = {}
            yall = {}
            yall[0] = pby03.tile([128, 4, 2, L], F8, name="yall0", tag="yall0")
            if not doB:
                nc.vector.memset(yall[0], 0.0)

            # ---------------- phase B helpers ----------------
            def load_qk(h):
                grp, hh = h // 4, h % 4
                qt = pb.tile([128, L], BF, name="qt", tag="qt")
                kt = pb.tile([128, L], BF, name="kt", tag="kt")
                for t, src in ((qt, qkrot[0][grp]), (kt, qkrot[1][grp])):
                    nc.sync.dma_start_transpose(
                        out=t, in_=src[:, hh * 128:(hh + 1) * 128])
                return qt, kt

            def emit_B_qc(h, qt, kt, qc):
                grp, hh = h // 4, h % 4
                nkt = 4 * qc + 4
                ypsum = P["psy"].tile([128, 512], F32, name="ypsum", tag="yp")
                dacc = pbd.tile([128, 512], F16, name="dacc", tag="dacc")
                ats = {}

                def emit_score(j):
                    m = j - 4 * qc
                    off = 128 * m if m > 0 else 0
                    w = 512 - off
                    sc = P["pss"].tile([128, 512], F32, name="sc", tag="sc")
                    nc.tensor.matmul(
                        sc[:, 0:w], kt[:, j * 128:(j + 1) * 128],
                        qt[:, qc * 512 + off:(qc + 1) * 512],
                        start=True, stop=True)
                    at = pba.tile([128, 512], F16, name="at", tag="at")
                    nc.scalar.activation(
                        out=at[:, 0:w], in_=sc[:, 0:w],
                        func=mybir.ActivationFunctionType.Exp,
                        scale=ALPHA, bias=ebias)
                    if m >= 0:
                        nc.vector.tensor_mul(at[:, 0:128], at[:, 0:128], trit)
                    ats[j] = (at, off, w)

                for j in range(min(la, nkt)):
                    emit_score(j)
                prev = None
                for j in range(nkt):
                    if j + la < nkt:
                        emit_score(j + la)
                    at, off, w = ats.pop(j)
                    nc.tensor.matmul(
                        ypsum[:, off:512],
                        vsb[grp][:, j, hh * 128:(hh + 1) * 128],
                        at[:, 0:w],
                        start=(j == 0), stop=(j == nkt - 1),
                        skip_group_check=True)
                    if j == 0:
                        prev = at
                    elif j == 1:
                        if qc == 0:
                            nc.vector.tensor_copy(
                                out=dacc[:, 0:128], in_=prev[:, 0:128])
                            nc.vector.tensor_add(
                                dacc[:, 128:512], prev[:, 128:512],
                                at[:, 0:w])
                        else:
                            nc.vector.tensor_add(dacc, prev, at)
                    else:
                        nc.vector.tensor_add(
                            dacc[:, off:512], dacc[:, off:512], at[:, 0:w])
                dpsum = P["psd"].tile([128, 512], F32, name="dpsum", tag="dp")
                nc.tensor.matmul(dpsum, ones128, dacc, start=True, stop=True)
                rbs = pbr.tile([128, 512], BF, name="rbs", tag="rbs")
                with nc.allow_low_precision("softmax recip bf16"):
                    nc.vector.reciprocal(out=rbs, in_=dpsum)
                qsl = slice(qc * 512, (qc + 1) * 512)
                ya = yall[grp]
                yb = pbv.tile([128, 512], BF, name="yb", tag="yb")
                nc.vector.tensor_mul(yb, ypsum, rbs)
                nc.gpsimd.tensor_copy(out=ya[:, hh, 1, qsl], in_=yb)
                yd = pbv.tile([128, 512], BF, name="yd", tag="yd")
                nc.gpsimd.tensor_sub(yd, yb, ya[:, hh, 1, qsl])
                nc.gpsimd.tensor_scalar_mul(ya[:, hh, 0, qsl], yd, 16.0)

            # ---------------- phase A scope + interleave ----------------
            with tc.tile_pool(name="pA", bufs=1) as pa, \
                 tc.tile_pool(name="pAw", bufs=2) as paw, \
                 tc.tile_pool(name="pAt", bufs=patb) as pat, \
                 tc.tile_pool(name="pAo", bufs=paob) as pao:
                xall = pa.tile([128, DT, 2, L], F8, name="xall", tag="xall")
                c_sb = pa.tile([128, LT, 256], BF, name="c_sb", tag="c_sb")
                s_sb = pa.tile([128, LT, 256], BF, name="s_sb", tag="s_sb")
                wch = {}

                def load_wch(c):
                    if c >= NCH or c in wch:
                        return
                    wa = paw.tile([128, DT, 512], F8, name="wchA", tag="wchA")
                    wb = paw.tile([128, DT, 2, 512], F8, name="wchB",
                                  tag="wchB")
                    wAr = wA[:, c * 512:(c + 1) * 512].rearrange(
                        "(d p) e -> p d e", p=128)
                    wBr = wB[:, c * 1024:(c + 1) * 1024].rearrange(
                        "(d p) e -> p d e", p=128)
                    wbf = wb.rearrange("p d t e -> p d (t e)")
                    for d4 in range(DT // 4):
                        sl = slice(4 * d4, 4 * d4 + 4)
                        nc.sync.dma_start(out=wa[:, sl, :], in_=wAr[:, sl, :])
                        nc.sync.dma_start(out=wbf[:, sl, :], in_=wBr[:, sl, :])
                    wch[c] = (wa, wb)

                def load_x():
                    wa = paw.tile([128, DT, 512], F8, name="wchA", tag="wchA")
                    wb = paw.tile([128, DT, 2, 512], F8, name="wchB",
                                  tag="wchB")
                    wAr = wA[:, 0:512].rearrange("(d p) e -> p d e", p=128)
                    wBr = wB[:, 0:1024].rearrange("(d p) e -> p d e", p=128)
                    wbf = wb.rearrange("p d t e -> p d (t e)")
                    xhr = xThi.rearrange("(d p) l -> p d l", p=128)
                    xlr = xTlo.rearrange("(d p) l -> p d l", p=128)
                    for d2 in range(DT // 2):
                        sl = slice(2 * d2, 2 * d2 + 2)
                        nc.sync.dma_start(out=xall[:, sl, 1, :],
                                          in_=xhr[:, sl, :])
                        nc.sync.dma_start(out=xall[:, sl, 0, :],
                                          in_=xlr[:, sl, :])
                        if d2 % 2 == 1:
                            sl4 = slice(2 * d2 - 2, 2 * d2 + 2)
                            nc.sync.dma_start(out=wa[:, sl4, :],
                                              in_=wAr[:, sl4, :])
                            nc.sync.dma_start(out=wbf[:, sl4, :],
                                              in_=wBr[:, sl4, :])
                            i2 = slice(d2 - 1, d2 + 1)
                            for t_sb, t_dr in ((c_sb, chalf), (s_sb, shalf)):
                                nc.sync.dma_start(
                                    out=t_sb[:, i2, :],
                                    in_=t_dr.rearrange("(i p) g -> p i g",
                                                       p=128)[:, i2, :])
                    # rope tables for the second half of the l range trail
                    for t_sb, t_dr in ((c_sb, chalf), (s_sb, shalf)):
                        nc.sync.dma_start(
                            out=t_sb[:, 8:LT, :],
                            in_=t_dr.rearrange("(i p) g -> p i g",
                                               p=128)[:, 8:LT, :])
                    wch[0] = (wa, wb)

                def emit_A_alpha(c, i):
                    wa, _ = wch[c]
                    ls = slice(i * 128, (i + 1) * 128)
                    pnat = P["psa"].tile([128, 512], F32, name="pnat",
                                         tag="pnat")
                    for d2 in range(DT // 2):
                        nc.tensor.matmul(
                            pnat,
                            xall[:, 2 * d2:2 * d2 + 2, 1, ls],
                            wa[:, 2 * d2:2 * d2 + 2, :],
                            start=(d2 == 0), stop=False, perf_mode=DR)
                    return pnat

                def emit_A_finish(c, i, pnat):
                    _, wb = wch[c]
                    if i == 8:
                        load_wch(c + 1)
                    ls = slice(i * 128, (i + 1) * 128)
                    for d in range(DT):
                        nc.tensor.matmul(
                            pnat,
                            xall[:, d, :, ls],
                            wb[:, d, :, :],
                            start=False, stop=(d == DT - 1), perf_mode=DR)
                    emit_A_rope(c, i, pnat)

                def emit_A_rope(c, i, pnat):
                    kind, grp = _chunk_kind(c)
                    ls = slice(i * 128, (i + 1) * 128)
                    if kind == "v":
                        nc.scalar.activation(
                            out=vsb[grp][:, i, :], in_=pnat,
                            func=mybir.ActivationFunctionType.Copy,
                            scale=1.0 / 1024.0)
                        return
                    # RoPE: per-head comps are permuted (evens | odds)
                    pv = pnat.rearrange("p (hh t z) -> p hh t z",
                                        hh=4, t=2, z=64)
                    x1 = pat.tile([128, 256], BF, name="x1", tag="x1")
                    nc.scalar.activation(
                        out=x1, in_=pv[:, :, 0, :],
                        func=mybir.ActivationFunctionType.Copy,
                        scale=1.0 / 1024.0)
                    x2 = pat.tile([128, 256], BF, name="x2", tag="x2")
                    nc.scalar.activation(
                        out=x2, in_=pv[:, :, 1, :],
                        func=mybir.ActivationFunctionType.Copy,
                        scale=1.0 / 1024.0)
                    ct = c_sb[:, i, :]
                    st = s_sb[:, i, :]
                    t1 = pat.tile([128, 256], BF, name="t1", tag="t1")
                    nc.vector.tensor_mul(t1, x1, ct)
                    t2 = pat.tile([128, 256], BF, name="t2", tag="t2")
                    nc.vector.tensor_mul(t2, x2, st)
                    t3 = pat.tile([128, 256], BF, name="t3", tag="t3")
                    nc.vector.tensor_mul(t3, x2, ct)
                    t4 = pat.tile([128, 256], BF, name="t4", tag="t4")
                    nc.vector.tensor_mul(t4, x1, st)
                    ro = pao.tile([128, 512], BF, name="ro", tag="ro")
                    rv = ro.rearrange("p (hh t z) -> p hh t z", hh=4, t=2, z=64)
                    nc.vector.tensor_sub(rv[:, :, 0, :], t1, t2)
                    nc.vector.tensor_add(rv[:, :, 1, :], t3, t4)
                    dst = qkrot[0 if kind == "q" else 1][grp]
                    nc.sync.dma_start(out=dst[ls, :], in_=ro)

                def emit_A_tile(c, i):
                    emit_A_finish(c, i, emit_A_alpha(c, i))

                def emit_A_dmajor(c, tiles):
                    # d-major across several open psum groups: every arriving
                    # x/w piece-group unlocks one alpha+2 betas per open tile
                    wa, wb = wch[c]
                    pns = [P["psa"].tile([128, 512], F32, name="pnat",
                                         tag="pnat") for _ in tiles]
                    for d2 in range(DT // 2):
                        for t, i in enumerate(tiles):
                            ls = slice(i * 128, (i + 1) * 128)
                            nc.tensor.matmul(
                                pns[t],
                                xall[:, 2 * d2:2 * d2 + 2, 1, ls],
                                wa[:, 2 * d2:2 * d2 + 2, :],
                                start=(d2 == 0), stop=False, perf_mode=DR)
                        for dd in (2 * d2, 2 * d2 + 1):
                            for t, i in enumerate(tiles):
                                ls = slice(i * 128, (i + 1) * 128)
                                nc.tensor.matmul(
                                    pns[t],
                                    xall[:, dd, :, ls],
                                    wb[:, dd, :, :],
                                    start=False, stop=(dd == DT - 1),
                                    perf_mode=DR)
                    for t, i in enumerate(tiles):
                        emit_A_rope(c, i, pns[t])

                # ---- emission: A(g0), then heads 0-3 x A(g1) ----
                if doA:
                    load_x()
                    # chunk 0 lead-in: 7 psum banks, d-major
                    with tc.tile_pool(name="psA0", bufs=8,
                                      space="PSUM") as psa0:
                        P["psa"] = psa0
                        emit_A_dmajor(0, range(8))
                        for i in range(8, LT):
                            emit_A_tile(0, i)
                with tc.tile_pool(name="psS", bufs=scb, space="PSUM") as pss, \
                     tc.tile_pool(name="psY", bufs=ypb, space="PSUM") as psy, \
                     tc.tile_pool(name="psD", bufs=1, space="PSUM") as psd:
                    P["pss"], P["psy"], P["psd"] = pss, psy, psd
                    with tc.tile_pool(name="psA", bufs=psab,
                                      space="PSUM") as psa:
                        P["psa"] = psa
                        if doA:
                            for c in (1, 2):
                                for i in range(LT):
                                    emit_A_tile(c, i)
                        g1 = [(c, i) for c in (3, 4, 5) for i in range(LT)] \
                            if doA else []
                        gi = 0
                        qts.clear()
                        for h in range(3 if doB else 0):
                            if h not in qts:
                                qts[h] = load_qk(h)
                            qt, kt = qts.pop(h)
                            for qc in range(QC):
                                emit_B_qc(h, qt, kt, qc)
                                if qc == 0 and h < 3 and doB:
                                    qts[h + 1] = load_qk(h + 1)
                                for _ in range(take):
                                    if gi < len(g1):
                                        emit_A_tile(*g1[gi])
                                        gi += 1
                        while gi < len(g1):
                            emit_A_tile(*g1[gi])
                            gi += 1

            # ---------------- heads 4-7 (qc-major) + phase C ----------
            with tc.tile_pool(name="pBy47", bufs=1) as pby47, \
                 tc.tile_pool(name="pCo", bufs=4) as pco, \
                 tc.tile_pool(name="psS3", bufs=scb, space="PSUM") as pss3, \
                 tc.tile_pool(name="psY3", bufs=ypb, space="PSUM") as psy3, \
                 tc.tile_pool(name="psD3", bufs=1, space="PSUM") as psd3, \
                 tc.tile_pool(name="psC", bufs=2, space="PSUM") as psc:
                P["pss"], P["psy"], P["psd"] = pss3, psy3, psd3
                qk47 = dict(qts)   # h3 was prefetched into the pb pool
                for h in range(4, NH if doB else 4):
                    grp, hh = h // 4, h % 4
                    qt = pby47.tile([128, L], BF, name=f"qt{h}", tag=f"qt{h}")
                    kt = pby47.tile([128, L], BF, name=f"kt{h}", tag=f"kt{h}")
                    for t, src in ((qt, qkrot[0][grp]), (kt, qkrot[1][grp])):
                        nc.sync.dma_start_transpose(
                            out=t, in_=src[:, hh * 128:(hh + 1) * 128])
                    qk47[h] = (qt, kt)
                yall[1] = pby47.tile([128, 4, 2, L], F8, name="yall1",
                                     tag="yall1")
                if not doB:
                    nc.vector.memset(yall[1], 0.0)
                woat = pby47.tile([128, NH, L], F8, name="woat", tag="woat")
                nc.sync.dma_start(
                    out=woat, in_=woA.rearrange("(dd p) e -> p dd e", p=128))
                wobt = pby47.tile([128, NH, 2, L], F8, name="wobt", tag="wobt")
                nc.sync.dma_start(
                    out=wobt.rearrange("p dd t e -> p dd (t e)"),
                    in_=woB.rearrange("(dd p) e -> p dd e", p=128))

                def emit_C(e, qc):
                    esl = slice(e * 128, (e + 1) * 128)
                    qsl = slice(qc * 512, (qc + 1) * 512)
                    op = psc.tile([128, 512], F32, name="op", tag="op")
                    for g in range(2):
                        for p2 in range(2):
                            dd = 4 * g + 2 * p2
                            nc.tensor.matmul(
                                op, woat[:, dd:dd + 2, esl],
                                yall[g][:, 2 * p2:2 * p2 + 2, 1, qsl],
                                start=(g == 0 and p2 == 0), stop=False,
                                perf_mode=DR)
                    for g in range(2):
                        for hh in range(4):
                            nc.tensor.matmul(
                                op, wobt[:, 4 * g + hh, :, esl],
                                yall[g][:, hh, :, qsl],
                                start=False, stop=(g == 1 and hh == 3),
                                perf_mode=DR)
                    ot = pco.tile([128, 512], BF, name="ot", tag="ot")
                    if (e + qc) % 2 == 0:
                        nc.scalar.activation(
                            out=ot, in_=op,
                            func=mybir.ActivationFunctionType.Copy,
                            scale=1.0 / 1024.0)
                    else:
                        nc.vector.tensor_scalar_mul(ot, op, 1.0 / 1024.0)
                    nc.sync.dma_start(
                        out=outT[e * 128:(e + 1) * 128,
                                 qc * 512:(qc + 1) * 512],
                        in_=ot)

                for qc in range(QC if doB else 0):
                    for h in range(3, NH):
                        emit_B_qc(h, qk47[h][0], qk47[h][1], qc)
                    if doC and qc > 0:
                        for e in range(DT):
                            emit_C(e, qc - 1)
                if doC:
                    for qc in ([3] if doB else range(QC)):
                        for e in range(DT):
                            emit_C(e, qc)
    nc.compile()
    return nc


_NC_CACHE = None


def _get_program():
    global _NC_CACHE
    if _NC_CACHE is None:
        _NC_CACHE = build_program()
    return _NC_CACHE


def _f8(a):
    return np.clip(np.asarray(a, np.float64), -240.0, 240.0).astype(F8NP)


# within each head's 128 comps: evens first, then odds
_PERM512 = np.concatenate(
    [np.concatenate([np.arange(h * 128, (h + 1) * 128, 2),
                     np.arange(h * 128 + 1, (h + 1) * 128, 2)])
     for h in range(4)])


def _host_inputs(x, w_qkv, w_o):
    inv = 1.0 / (ROPE_BASE ** (np.arange(0, HD, 2, dtype=np.float64) / HD))
    ang = np.arange(L, dtype=np.float64)[:, None] * inv[None, :]
    chalf = np.tile(np.cos(ang), (1, 4)).astype(BF16)          # [L, 256]
    shalf = np.tile(np.sin(ang), (1, 4)).astype(BF16)
    p = np.arange(128)[:, None]
    f = np.arange(128)[None, :]
    tri = (p <= f).astype(np.float16)                          # [128, 128]

    xhi_all, xlo_all = [], []
    for b in range(B):
        xb = np.asarray(x[b], np.float64)                      # [L, D]
        xh = _f8(xb)
        xl = _f8(16.0 * (xb - xh.astype(np.float64)))
        xhi_all.append(np.ascontiguousarray(xh.T))
        xlo_all.append(np.ascontiguousarray(xl.T))

    in_maps = []
    for c in range(8):
        b, g = c % 4, c // 4
        qr = w_qkv[g * DL:(g + 1) * DL]
        kr = w_qkv[D + g * DL:D + (g + 1) * DL]
        vr = w_qkv[2 * D + g * DL:2 * D + (g + 1) * DL]
        qp = qr[np.concatenate([_PERM512, 512 + _PERM512])]
        kp = kr[np.concatenate([_PERM512, 512 + _PERM512])]
        wloc = np.concatenate([qp[:512], kp[:512], vr[:512],
                               qp[512:], kp[512:], vr[512:]],
                              axis=0).astype(np.float64)       # [3DL, D]
        w64h = _f8(64.0 * wloc)
        w1024h = (w64h.astype(np.float64) * 16.0).astype(F8NP)  # exact shift
        w16l = _f8(16.0 * (64.0 * wloc - w64h.astype(np.float64)))
        wAh = np.ascontiguousarray(w1024h.T)                    # [D, 3DL]
        w64hT, w16lT = w64h.T, w16l.T                           # [D, 3DL]
        wBh = np.ascontiguousarray(np.concatenate(
            [np.stack([w64hT[:, c2 * 512:(c2 + 1) * 512],
                       w16lT[:, c2 * 512:(c2 + 1) * 512]],
                      axis=1).reshape(D, 1024)
             for c2 in range(NCH)], axis=1))                    # [D, 6DL]
        woT64 = 64.0 * w_o[:, g * DL:(g + 1) * DL].T.astype(np.float64)
        wo64h = _f8(woT64)
        woAh = np.ascontiguousarray(
            (wo64h.astype(np.float64) * 16.0).astype(F8NP))      # [DL, L]
        wo16l = _f8(16.0 * (woT64 - wo64h.astype(np.float64)))
        woBh = np.ascontiguousarray(
            np.concatenate([wo64h[:, None, :], wo16l[:, None, :]],
                           axis=1).reshape(DL, 2 * L))
        in_maps.append({
            "xThi": xhi_all[b],
            "xTlo": xlo_all[b],
            "wA": wAh,
            "wB": wBh,
            "woA": woAh,
            "woB": woBh,
            "chalf": chalf,
            "shalf": shalf,
            "tri": tri,
        })
    return in_maps


def kernel(x, w_qkv, w_o, _trace=False):
    x = np.asarray(x, dtype=np.float32)
    w_qkv = np.asarray(w_qkv, dtype=np.float32)
    w_o = np.asarray(w_o, dtype=np.float32)
    nc = _get_program()
    in_maps = _host_inputs(x, w_qkv, w_o)
    res = run_bass_kernel_spmd(nc, in_maps, core_ids=list(range(8)),
                               trace=_trace)
    kernel.last_result = res
    parts = [r["outT"].astype(np.float32) for r in res.results]
    out = np.empty((B, L, D), dtype=np.float32)
    for b in range(B):
        out[b] = (parts[b] + parts[b + 4]).T
    return out
